# revision 52
# baseline (speedup 1.0000x reference)
"""GQA causal attention (B=1, S=2048, D=4096, H=32, KV=8) on 8 trn2 cores.

Strategy: tensor-parallel over heads. Core i owns q-heads 4i..4i+3 and
kv-head i. Host pre-transposes weights/x so every matmul contracts along
the partition dim, and pre-permutes wq/wk rows (even|odd interleave ->
[evens;odds]) so RoPE becomes partition-aligned elementwise math.
Attention is computed head-locally in a scores^T [t, s] layout; softmax
sums are accumulated on the vector engine and reduced across partitions
with one gpsimd partition_all_reduce per (head, block) — no norm matmul
stream on the PE. Exp runs on 1024-wide pairs of score tiles to amortize
the activation engine's per-instruction overhead. After each 512-row
block of attention, the local out-projection shard is computed and a
bf16 ReduceScatter is fired as soon as each row-chunk completes, so the
collectives overlap compute. Host concatenates the per-core shards.

Matmul operands are bf16; accumulation, softmax and RoPE math are fp32.
"""

import sys

import numpy as np

sys.path.insert(0, "/opt/trn_rl_repo")

import ml_dtypes  # noqa: E402

import concourse.bass as bass  # noqa: E402
from concourse import bacc  # noqa: E402
from concourse import bass_isa  # noqa: E402
import concourse.mybir as mybir  # noqa: E402
import concourse.tile as tile  # noqa: E402
from concourse.bass_utils import run_bass_kernel_spmd  # noqa: E402

F32 = mybir.dt.float32
BF16 = mybir.dt.bfloat16
NPBF = ml_dtypes.bfloat16

B, S, D = 1, 2048, 4096
H, KV, HD = 32, 8, 128
NCORES = 8
HPC = H // NCORES  # q heads per core = 4
EQ = HPC * HD  # 512 local q features
NE = HPC + 2  # e-tiles per core: 4 q + 1 k + 1 v
SB = 512  # attention s block
NSB = S // SB  # 4
XSB = 512  # phase-1 s sub-block (matmul moving dim)
NXSB = S // XSB  # 4
DO = D // 128  # 32 contraction tiles for projections
TT = S // 128  # 16 t-tiles
RG = [list(range(NCORES))]
CH = [(12, 14), (14, 16), (8, 12), (4, 8), (0, 2), (2, 4)]


def build():
    nc = bacc.Bacc("TRN2", target_bir_lowering=False)
    # partition-major: [p, sb, do, c] so each phase-1 piece is a 16KB
    # contiguous run per partition (few, large DMA descriptors)
    xt = nc.dram_tensor("xt", [128, NXSB * DO * XSB], BF16,
                        kind="ExternalInput")
    # partition-major layout: [p, et, do, c] so each per-et DMA moves 8KB
    # contiguous per partition (line-rate) in PE consumption order
    wqkvt = nc.dram_tensor("wqkvt", [128, NE * DO * 128], BF16,
                           kind="ExternalInput")
    wot = nc.dram_tensor("wot", [EQ, D], BF16, kind="ExternalInput")
    cc = nc.dram_tensor("cc", [128, S], F32, kind="ExternalInput")
    ss = nc.dram_tensor("ss", [128, S], F32, kind="ExternalInput")
    masks2 = nc.dram_tensor("masks2", [2, 128, 2 * SB], BF16,
                            kind="ExternalInput")
    ident = nc.dram_tensor("ident", [128, 128], BF16, kind="ExternalInput")
    out = nc.dram_tensor("out", [NSB * 64, D], BF16, kind="ExternalOutput")

    xt_t = xt[:].rearrange("p (sb do c) -> p sb do c", sb=NXSB, do=DO)
    w_t = wqkvt[:].rearrange("p (et do c) -> p et (do c)", et=NE, do=DO)

    with tile.TileContext(nc) as tc:
        with tc.tile_pool(name="dram", bufs=1, space="DRAM") as dram, \
                tc.tile_pool(name="pqkv", bufs=1) as pqkv:
            rs_in = [dram.tile([(t1 - t0) * 128, D], BF16, name=f"rsi{ci}")
                     for ci, (t0, t1) in enumerate(CH)]
            rs_out = [dram.tile([(t1 - t0) * 16, D], BF16, name=f"rso{ci}")
                      for ci, (t0, t1) in enumerate(CH)]
            qe = [pqkv.tile([128, S], BF16, name=f"qe{et}", tag=f"qe{et}")
                  for et in range(NE)]
            # v in natural [t, hd] tiles (filled by per-sb PE transposes)
            vn = pqkv.tile([128, TT, HD], BF16, tag="vn")
            idt = pqkv.tile([128, 128], BF16, tag="idt")
            nc.scalar.dma_start(idt, ident[:])

            # ---------------- Phase 1: fused QKV projection + RoPE ----------
            with tc.tile_pool(name="p1w", bufs=1) as p1w, \
                    tc.tile_pool(name="p1x", bufs=2) as p1x, \
                    tc.tile_pool(name="p1t", bufs=1) as p1t, \
                    tc.tile_pool(name="p1ps", bufs=3, space="PSUM") as p1ps:
                w = p1w.tile([128, NE, DO, 128], BF16)
                # weights issued et-major so the PE's (sb0, et) groups are
                # fed in exactly the order they're consumed
                wv_ = w_t[:].rearrange("p et (do c) -> p et do c", do=DO)
                for dh in range(2):
                    dsl = slice(dh * (DO // 2), (dh + 1) * (DO // 2))
                    nc.scalar.dma_start(w[:, 0, dsl, :], wv_[:, 0, dsl, :])
                for et in range(1, NE):
                    nc.scalar.dma_start(
                        w[:, et].rearrange("p do c -> p (do c)"), w_t[:, et])
                # x for sb0/sb1 prefetched in pieces (first pieces smallest so
                # the PE's first accumulation group starts ASAP)
                xtiles, ctiles, stiles = [], [], []
                for sb in range(2):
                    ssl = slice(sb * XSB, (sb + 1) * XSB)
                    xtile = p1x.tile([128, DO, XSB], BF16, tag="x")
                    chunks = (8, 8, 8, 8) if sb == 0 else (16, 16)
                    d0 = 0
                    for nd in chunks:
                        dsl = slice(d0, d0 + nd)
                        nc.sync.dma_start(
                            xtile[:, dsl, :], xt_t[:, sb, dsl, :])
                        d0 += nd
                    cct = p1x.tile([128, XSB], F32, tag="cc")
                    sst = p1x.tile([128, XSB], F32, tag="ss")
                    nc.sync.dma_start(cct, cc[:][:, ssl])
                    nc.sync.dma_start(sst, ss[:][:, ssl])
                    xtiles.append(xtile)
                    ctiles.append(cct)
                    stiles.append(sst)
                for sb in range(NXSB):
                    ssl = slice(sb * XSB, (sb + 1) * XSB)
                    if sb < 2:
                        xtile, cct, sst = xtiles[sb], ctiles[sb], stiles[sb]
                    else:
                        xtile = p1x.tile([128, DO, XSB], BF16, tag="x")
                        nc.sync.dma_start(xtile, xt_t[:, sb])
                        cct = p1x.tile([128, XSB], F32, tag="cc")
                        sst = p1x.tile([128, XSB], F32, tag="ss")
                        nc.sync.dma_start(cct, cc[:][:, ssl])
                        nc.sync.dma_start(sst, ss[:][:, ssl])
                    atile = p1t.tile([128, HPC + 1, XSB], F32, tag="at")
                    btile = p1t.tile([128, HPC + 1, XSB], F32, tag="bt")
                    bsw = p1t.tile([128, (HPC + 1) * XSB], F32, tag="bsw")
                    bt2 = btile.rearrange("p e s -> p (e s)")
                    bs3 = bsw.rearrange("p (e s) -> p e s", s=XSB)
                    last = sb == NXSB - 1
                    # in the last sub-block, compute k first and finish its
                    # RoPE early so attention's first scores aren't gated on
                    # the whole RoPE tail
                    ets = (4, 0, 1, 2, 3, 5) if last else (0, 1, 2, 3, 4, 5)
                    for et in ets:
                        ps = p1ps.tile([128, XSB], F32, tag="ps")
                        for do in range(DO):
                            nc.tensor.matmul(
                                ps,
                                w[:, et, do, :],
                                xtile[:, do, :],
                                start=(do == 0),
                                stop=(do == DO - 1),
                            )
                        if et < HPC + 1:
                            # t*cos and t*sin halves; swap+add finishes RoPE
                            nc.vector.tensor_mul(atile[:, et, :], ps, cct)
                            nc.vector.tensor_mul(btile[:, et, :], ps, sst)
                            if last:
                                # per-et swap+add so each qe slice is ready
                                # ~2us after its matmuls, shrinking the
                                # phase-1 -> attention bubble
                                esl_ = slice(et * XSB, (et + 1) * XSB)
                                nc.gpsimd.dma_start(
                                    bsw[:64, esl_], bt2[64:, esl_])
                                nc.gpsimd.dma_start(
                                    bsw[64:, esl_], bt2[:64, esl_])
                                nc.vector.tensor_add(
                                    qe[et][:, ssl], atile[:, et, :],
                                    bs3[:, et, :])
                        else:
                            nc.scalar.copy(qe[et][:, ssl], ps)
                    if not last:
                        # half-swap of btile on the (idle) gpsimd DMA queue
                        nc.gpsimd.dma_start(bsw[:64, :], bt2[64:, :])
                        nc.gpsimd.dma_start(bsw[64:, :], bt2[:64, :])
                        for et in range(HPC + 1):
                            nc.vector.tensor_add(
                                qe[et][:, ssl], atile[:, et, :],
                                bs3[:, et, :])
                    # transpose this sb's four v tiles into [t, hd] layout
                    for i in range(4):
                        tt = 4 * sb + i
                        pst = p1ps.tile([128, 128], BF16, tag="tr", bufs=2)
                        nc.tensor.transpose(
                            pst, qe[HPC + 1][:, tt * 128:(tt + 1) * 128], idt)
                        nc.scalar.copy(vn[:, tt, :], pst)

            # ------- Phase 2+3: attention blocks + interleaved out-proj -----
            with tc.tile_pool(name="p3w", bufs=1) as p3w, \
                    tc.tile_pool(name="p2c", bufs=1) as p2c, \
                    tc.tile_pool(name="p2e", bufs=8) as p2e, \
                    tc.tile_pool(name="p2a", bufs=2) as p2a, \
                    tc.tile_pool(name="p2t", bufs=2) as p2t, \
                    tc.tile_pool(name="p3o", bufs=2) as p3o, \
                    tc.tile_pool(name="psS", bufs=2, space="PSUM") as psS, \
                    tc.tile_pool(name="pav", bufs=2, space="PSUM") as pavp, \
                    tc.tile_pool(name="pop", bufs=2, space="PSUM") as popp:
                mt = p2c.tile([128, 2, 2 * SB], BF16)
                nc.sync.dma_start(mt, masks2[:].rearrange("m p s -> p m s"))
                w3 = p3w.tile([128, HPC, D], BF16)
                nc.sync.dma_start(
                    w3, wot[:].rearrange("(eo p) d -> p eo d", p=128))
                ao = [p2c.tile([128, S], BF16, name=f"ao{h}", tag=f"ao{h}")
                      for h in range(HPC)]

                FIRED = {"tiles": set(), "fired": set()}

                def _flush(item):
                    fh, fbsl, fpau, fnrmb = item
                    nc.vector.reciprocal_approx_fast(fnrmb, fnrmb)
                    nc.vector.tensor_mul(ao[fh][:, fbsl], fpau, fnrmb)

                def emit_attn(b):
                    bsl = slice(b * SB, (b + 1) * SB)
                    npair = 2 * b + 2
                    porder = list(range(npair))
                    deferred = []
                    for h in range(HPC):
                        # renorm of head h-2 runs now: its partition reduce is
                        # long done, so the DVE never head-of-line blocks
                        if len(deferred) >= 2:
                            _flush(deferred.pop(0))
                        pa = pavp.tile([128, SB], F32, tag="pav")
                        partials = []
                        for pi, p in enumerate(porder):
                            j0, j1 = 2 * p, 2 * p + 1
                            psc = psS.tile([128, 2 * SB], F32, tag="sc")
                            nc.tensor.matmul(
                                psc[:, :SB],
                                qe[HPC][:, j0 * 128:(j0 + 1) * 128],
                                qe[h][:, bsl],
                                start=True, stop=True)
                            nc.tensor.matmul(
                                psc[:, SB:],
                                qe[HPC][:, j1 * 128:(j1 + 1) * 128],
                                qe[h][:, bsl],
                                start=True, stop=True)
                            ex = p2e.tile([128, 2 * SB], BF16, tag="ex")
                            nc.scalar.activation(
                                ex, psc, mybir.ActivationFunctionType.Exp)
                            if p >= 2 * b:
                                nc.vector.tensor_mul(
                                    ex, ex, mt[:, p - 2 * b, :])
                            # bf16 pair-sum feeding the softmax-denominator
                            # tree (2x DVE rate; depth<=4 so rounding is tiny)
                            tp = p2t.tile([128, SB], BF16, tag="tp", bufs=12)
                            nc.vector.tensor_add(tp, ex[:, :SB], ex[:, SB:])
                            partials.append(tp)
                            nc.tensor.matmul(
                                pa, vn[:, j0, :], ex[:, :SB],
                                start=(pi == 0), stop=False)
                            nc.tensor.matmul(
                                pa, vn[:, j1, :], ex[:, SB:],
                                start=False, stop=(pi == npair - 1))
                        while len(partials) > 2:
                            nxt = []
                            for k in range(0, len(partials) - 1, 2):
                                u = p2t.tile([128, SB], BF16, tag="tp",
                                             bufs=12)
                                nc.vector.tensor_add(
                                    u, partials[k], partials[k + 1])
                                nxt.append(u)
                            if len(partials) % 2:
                                nxt.append(partials[-1])
                            partials = nxt
                        acc = p2a.tile([128, SB], F32, tag="acc")
                        nc.vector.tensor_add(acc, partials[0], partials[1])
                        nrmb = p2a.tile([128, SB], F32, tag="nrm", bufs=4)
                        nc.gpsimd.partition_all_reduce(
                            nrmb, acc, 128, bass_isa.ReduceOp.add)
                        # evacuate pa so its PSUM bank recycles immediately
                        pau = p2a.tile([128, SB], F32, tag="pau", bufs=4)
                        nc.vector.tensor_copy(pau, pa)
                        deferred.append((h, bsl, pau, nrmb))
                    for item in deferred:
                        _flush(item)

                def emit_oproj(b):
                    # out-projection for this block's four s-tiles
                    done_tiles = FIRED["tiles"]
                    for stl_i in range(4):
                        st = b * 4 + stl_i
                        st0 = st * 128
                        ci = next(i for i, (t0, t1) in enumerate(CH)
                                  if t0 <= st < t1)
                        t0, t1 = CH[ci]
                        riv = rs_in[ci].rearrange("(t p) d -> p t d", p=128)
                        osb = p3o.tile([128, D], BF16, tag="osb")
                        for db in range(D // SB):
                            po = popp.tile([128, SB], F32, tag="pop")
                            for hh in range(HPC):
                                nc.tensor.matmul(
                                    po,
                                    ao[hh][:, st0:st0 + 128],
                                    w3[:, hh, db * SB:(db + 1) * SB],
                                    start=(hh == 0), stop=(hh == HPC - 1))
                            nc.scalar.copy(
                                osb[:, db * SB:(db + 1) * SB], po)
                        nc.sync.dma_start(riv[:, st - t0, :], osb)
                        done_tiles.add(st)
                        for ci2, (u0, u1) in enumerate(CH):
                            if ci2 not in FIRED["fired"] and all(
                                    t in done_tiles for t in range(u0, u1)):
                                FIRED["fired"].add(ci2)
                                nc.gpsimd.collective_compute(
                                    "ReduceScatter",
                                    mybir.AluOpType.add,
                                    ins=[rs_in[ci2].opt()],
                                    outs=[rs_out[ci2].opt()],
                                    replica_groups=RG,
                                )

                # Virtual ready-times pin the static schedule to this phase
                # order — without them the scheduler hoists out-proj
                # LDWEIGHTS/ship DMAs into earlier phases where their guards
                # stall the whole engine stream. Out-proj directly follows
                # each block so the ReduceScatter pipeline starts as early
                # as possible (it is the second-half critical path).
                emit_attn(3)
                with tc.tile_wait_until(1):
                    emit_oproj(3)
                with tc.tile_wait_until(2):
                    emit_attn(2)
                with tc.tile_wait_until(3):
                    emit_oproj(2)
                with tc.tile_wait_until(4):
                    emit_attn(1)
                with tc.tile_wait_until(5):
                    emit_oproj(1)
                with tc.tile_wait_until(6):
                    emit_attn(0)
                with tc.tile_wait_until(7):
                    emit_oproj(0)
                # tail: ship RS shards (bf16) straight to the output; host
                # upcasts to fp32
                # ship on the scalar HWDGE queue: the gpsimd queue is strict
                # FIFO and a ship DMA there would block the partition
                # reduces queued behind it for a whole collective
                orow = 0
                with tc.tile_wait_until(8):
                    for ci, (t0, t1) in enumerate(CH):
                        nr = (t1 - t0) * 16
                        nc.sync.dma_start(
                            out[:][orow:orow + nr, :], rs_out[ci][:])
                        orow += nr
    nc.compile()
    return nc


_CACHE = {}


def _get_program():
    if "nc" not in _CACHE:
        _CACHE["nc"] = build()
    return _CACHE["nc"]


def _host_prep(x, freqs_cos, freqs_sin, wq, wk, wv, wo):
    x2 = np.ascontiguousarray(np.asarray(x, np.float32).reshape(S, D))
    # partition-major repack: [p, sb, do, c] = xT[do*128+p, sb*512+c]
    xT = np.ascontiguousarray(
        x2.T.reshape(DO, 128, NXSB, XSB).transpose(1, 2, 0, 3)
        .reshape(128, NXSB * DO * XSB)).astype(NPBF)
    # even|odd -> [evens;odds] row permutation per head (RoPE partition split)
    perm1 = np.concatenate([np.arange(0, HD, 2), np.arange(1, HD, 2)])
    permq = (np.arange(H)[:, None] * HD + perm1[None, :]).reshape(-1)
    permk = (np.arange(KV)[:, None] * HD + perm1[None, :]).reshape(-1)
    scale = np.float32(1.0 / np.sqrt(HD))
    wq_p = np.asarray(wq, np.float32)[permq] * scale
    wk_p = np.asarray(wk, np.float32)[permk]
    wv32 = np.asarray(wv, np.float32)
    wo32 = np.asarray(wo, np.float32)
    cosT = np.asarray(freqs_cos, np.float32).T
    sinT = np.asarray(freqs_sin, np.float32).T
    ccb = np.ascontiguousarray(np.concatenate([cosT, cosT], 0))
    ssb = np.ascontiguousarray(np.concatenate([sinT, -sinT], 0))
    tp = np.arange(128, dtype=np.int64)[:, None]
    sf = np.arange(SB, dtype=np.int64)[None, :]
    masks = [(sf >= tp + 128 * m).astype(NPBF) for m in range(HPC)]
    masks2 = np.stack([np.concatenate([masks[0], masks[1]], 1),
                       np.concatenate([masks[2], masks[3]], 1)], 0)
    ident = np.eye(128, dtype=NPBF)

    in_maps = []
    for i in range(NCORES):
        wqkv = np.concatenate(
            [wq_p[i * EQ:(i + 1) * EQ],
             wk_p[i * HD:(i + 1) * HD],
             wv32[i * HD:(i + 1) * HD]], 0)  # [768, 4096]
        # partition-major repack: [p, et, do, c] with
        # element = wqkv[et*128 + c, do*128 + p]
        wqkvt = np.ascontiguousarray(
            wqkv.reshape(NE, 128, DO, 128).transpose(3, 0, 2, 1)
            .reshape(128, NE * DO * 128)).astype(NPBF)
        wot = np.ascontiguousarray(
            wo32[:, i * EQ:(i + 1) * EQ].T).astype(NPBF)  # [512, 4096]
        in_maps.append(dict(xt=xT, wqkvt=wqkvt, wot=wot, cc=ccb, ss=ssb,
                            masks2=masks2, ident=ident))
    return in_maps


def _run(in_maps, trace=False):
    nc = _get_program()
    return run_bass_kernel_spmd(
        nc, in_maps, core_ids=list(range(NCORES)), trace=trace)


CH_HOST = list(CH)


def _assemble(res):
    full = np.empty((S, D), np.float32)
    for r in range(NCORES):
        shard = np.asarray(res.results[r]["out"]).astype(np.float32)
        orow = 0
        for (t0, t1) in CH_HOST:
            nr = (t1 - t0) * 16
            full[t0 * 128 + r * nr: t0 * 128 + (r + 1) * nr, :] = \
                shard[orow:orow + nr, :]
            orow += nr
    return full.reshape(B, S, D)


def kernel(x, freqs_cos, freqs_sin, wq, wk, wv, wo):
    in_maps = _host_prep(x, freqs_cos, freqs_sin, wq, wk, wv, wo)
    res = _run(in_maps, trace=False)
    return _assemble(res)


def _build_sharded():
    """Mirror of bass2jax.run_bass_via_pjrt's multi-core path, split so the
    jitted callable and device-resident inputs can be reused for timing."""
    import jax
    from jax.experimental.shard_map import shard_map
    from jax.sharding import Mesh, PartitionSpec

    import concourse.mybir as mb
    from concourse import bass2jax

    nc = _get_program()
    bass2jax.install_neuronx_cc_hook()
    part_name = (nc.partition_id_tensor.name
                 if nc.partition_id_tensor else None)
    in_names, out_names, out_avals, zero_outs = [], [], [], []
    for alloc in nc.m.functions[0].allocations:
        if not isinstance(alloc, mb.MemoryLocationSet):
            continue
        name = alloc.memorylocations[0].name
        if alloc.kind == "ExternalInput":
            if name != part_name:
                in_names.append(name)
        elif alloc.kind == "ExternalOutput":
            out_names.append(name)
            shape = tuple(alloc.tensor_shape)
            dtype = mb.dt.np(alloc.dtype)
            out_avals.append(jax.core.ShapedArray(shape, dtype))
            zero_outs.append(np.zeros(shape, dtype))
    n_params = len(in_names)
    all_names = in_names + out_names
    if part_name is not None:
        all_names = all_names + [part_name]

    def _body(*args):
        operands = list(args)
        if part_name is not None:
            operands.append(bass2jax.partition_id_tensor())
        outs = bass2jax._bass_exec_p.bind(
            *operands,
            out_avals=tuple(out_avals),
            in_names=tuple(all_names),
            out_names=tuple(out_names),
            lowering_input_output_aliases=(),
            sim_require_finite=True,
            sim_require_nnan=True,
            nc=nc,
        )
        return tuple(outs)

    devices = jax.devices()[:NCORES]
    mesh = Mesh(np.asarray(devices), ("core",))
    n_outs = len(out_names)
    sharded = jax.jit(
        shard_map(
            _body, mesh=mesh,
            in_specs=(PartitionSpec("core"),) * (n_params + n_outs),
            out_specs=(PartitionSpec("core"),) * n_outs,
            check_rep=False,
        ),
        donate_argnums=tuple(range(n_params, n_params + n_outs)),
        keep_unused=True,
    )
    return sharded, in_names, out_names, out_avals, zero_outs, mesh


def kernel_profiled(x, freqs_cos, freqs_sin, wq, wk, wv, wo, iters=12):
    """Returns (output, per-execution wall ns). Times repeated on-device
    executions with inputs pre-placed on the devices."""
    import time

    import jax
    from jax.sharding import NamedSharding, PartitionSpec

    in_maps = _host_prep(x, freqs_cos, freqs_sin, wq, wk, wv, wo)
    sharded, in_names, out_names, out_avals, zero_outs, mesh = _build_sharded()
    spec = NamedSharding(mesh, PartitionSpec("core"))
    concat_in = [
        jax.device_put(
            np.concatenate([in_maps[c][n] for c in range(NCORES)], axis=0),
            spec)
        for n in in_names
    ]

    def zeros():
        return [
            jax.device_put(
                np.zeros((NCORES * z.shape[0], *z.shape[1:]), z.dtype), spec)
            for z in zero_outs
        ]

    out_arrs = sharded(*concat_in, *zeros())  # warmup & result
    jax.block_until_ready(out_arrs)
    result = [np.asarray(a) for a in out_arrs]

    zsets = [zeros() for _ in range(iters)]
    jax.block_until_ready(zsets)
    t0 = time.perf_counter()
    last = None
    for zs in zsets:
        last = sharded(*concat_in, *zs)
    jax.block_until_ready(last)
    t1 = time.perf_counter()
    per_iter_ns = (t1 - t0) / iters * 1e9

    res_maps = [
        {n: result[i].reshape(NCORES, *out_avals[i].shape)[c]
         for i, n in enumerate(out_names)}
        for c in range(NCORES)
    ]

    class _R:
        results = res_maps

    return _assemble(_R), per_iter_ns


def _enable_ntff_hook():
    """Synthesize antenv.axon_hooks (absent in this image) and register the
    ctypes NTFF profile hook so run_bass_kernel_spmd(trace=True) works."""
    import sys as _sys
    import types as _types

    if "antenv.axon_hooks" in _sys.modules:
        return
    import antenv  # noqa: F401
    mod = _types.ModuleType("antenv.axon_hooks")
    mod._hook = None

    def set_axon_ntff_profile_hook(h):
        mod._hook = h

    def get_axon_ntff_profile_hook():
        return mod._hook

    mod.set_axon_ntff_profile_hook = set_axon_ntff_profile_hook
    mod.get_axon_ntff_profile_hook = get_axon_ntff_profile_hook
    _sys.modules["antenv.axon_hooks"] = mod
    antenv.axon_hooks = mod
    from trn_agent_boot.trn_boot import _ntff_profile_via_ctypes
    hook = _ntff_profile_via_ctypes("/opt/axon/libaxon_pjrt.so")
    if hook is not None:
        mod.set_axon_ntff_profile_hook(hook)
    # uploads need a fish bucket this container lacks; neuter them
    import concourse.bass_utils as _bu
    _bu.upload_artifacts = lambda tmpdir: f"local:{tmpdir}"


def kernel_traced(x, freqs_cos, freqs_sin, wq, wk, wv, wo, tmpdir=None):
    """Run once with NTFF tracing; returns (output, BassKernelResults)."""
    _enable_ntff_hook()
    in_maps = _host_prep(x, freqs_cos, freqs_sin, wq, wk, wv, wo)
    nc = _get_program()
    res = run_bass_kernel_spmd(
        nc, in_maps, core_ids=list(range(NCORES)), trace=True, tmpdir=tmpdir)
    return _assemble(res), res


# revision 54
# speedup vs baseline: 1.0501x; 1.0501x over previous
"""GQA causal attention (B=1, S=2048, D=4096, H=32, KV=8) on 8 trn2 cores.

Strategy: tensor-parallel over heads. Core i owns q-heads 4i..4i+3 and
kv-head i. Host pre-transposes weights/x so every matmul contracts along
the partition dim, and pre-permutes wq/wk rows (even|odd interleave ->
[evens;odds]) so RoPE becomes partition-aligned elementwise math.
Attention is computed head-locally in a scores^T [t, s] layout; softmax
sums are accumulated on the vector engine and reduced across partitions
with one gpsimd partition_all_reduce per (head, block) — no norm matmul
stream on the PE. Exp runs on 1024-wide pairs of score tiles to amortize
the activation engine's per-instruction overhead. After each 512-row
block of attention, the local out-projection shard is computed and a
bf16 ReduceScatter is fired as soon as each row-chunk completes, so the
collectives overlap compute. Host concatenates the per-core shards.

Matmul operands are bf16; accumulation, softmax and RoPE math are fp32.
"""

import sys

import numpy as np

sys.path.insert(0, "/opt/trn_rl_repo")

import ml_dtypes  # noqa: E402

import concourse.bass as bass  # noqa: E402
from concourse import bacc  # noqa: E402
from concourse import bass_isa  # noqa: E402
import concourse.mybir as mybir  # noqa: E402
import concourse.tile as tile  # noqa: E402
from concourse.bass_utils import run_bass_kernel_spmd  # noqa: E402

F32 = mybir.dt.float32
BF16 = mybir.dt.bfloat16
NPBF = ml_dtypes.bfloat16

B, S, D = 1, 2048, 4096
H, KV, HD = 32, 8, 128
NCORES = 8
HPC = H // NCORES  # q heads per core = 4
EQ = HPC * HD  # 512 local q features
NE = HPC + 2  # e-tiles per core: 4 q + 1 k + 1 v
SB = 512  # attention s block
NSB = S // SB  # 4
XSB = 512  # phase-1 s sub-block (matmul moving dim)
NXSB = S // XSB  # 4
DO = D // 128  # 32 contraction tiles for projections
TT = S // 128  # 16 t-tiles
RG = [list(range(NCORES))]
CH = [(12, 14), (14, 16), (8, 12), (4, 8), (0, 2), (2, 4)]


def build():
    nc = bacc.Bacc("TRN2", target_bir_lowering=False)
    # partition-major: [p, sb, do, c] so each phase-1 piece is a 16KB
    # contiguous run per partition (few, large DMA descriptors)
    xt = nc.dram_tensor("xt", [128, NXSB * DO * XSB], BF16,
                        kind="ExternalInput")
    # partition-major layout: [p, et, do, c] so each per-et DMA moves 8KB
    # contiguous per partition (line-rate) in PE consumption order
    wqkvt = nc.dram_tensor("wqkvt", [128, NE * DO * 128], BF16,
                           kind="ExternalInput")
    wot = nc.dram_tensor("wot", [EQ, D], BF16, kind="ExternalInput")
    cc = nc.dram_tensor("cc", [128, S], F32, kind="ExternalInput")
    ss = nc.dram_tensor("ss", [128, S], F32, kind="ExternalInput")
    masks2 = nc.dram_tensor("masks2", [2, 128, 2 * SB], BF16,
                            kind="ExternalInput")
    ident = nc.dram_tensor("ident", [128, 128], BF16, kind="ExternalInput")
    out = nc.dram_tensor("out", [NSB * 64, D], BF16, kind="ExternalOutput")

    xt_t = xt[:].rearrange("p (sb do c) -> p sb do c", sb=NXSB, do=DO)
    w_t = wqkvt[:].rearrange("p (et do c) -> p et (do c)", et=NE, do=DO)

    with tile.TileContext(nc) as tc:
        with tc.tile_pool(name="dram", bufs=1, space="DRAM") as dram, \
                tc.tile_pool(name="pqkv", bufs=1) as pqkv:
            rs_in = [dram.tile([(t1 - t0) * 128, D], BF16, name=f"rsi{ci}")
                     for ci, (t0, t1) in enumerate(CH)]
            rs_out = [dram.tile([(t1 - t0) * 16, D], BF16, name=f"rso{ci}")
                      for ci, (t0, t1) in enumerate(CH)]
            qe = [pqkv.tile([128, S], BF16, name=f"qe{et}", tag=f"qe{et}")
                  for et in range(NE)]
            # v in natural [t, hd] tiles (filled by per-sb PE transposes)
            vn = pqkv.tile([128, TT, HD], BF16, tag="vn")
            idt = pqkv.tile([128, 128], BF16, tag="idt")
            nc.scalar.dma_start(idt, ident[:])

            # ---------------- Phase 1: fused QKV projection + RoPE ----------
            with tc.tile_pool(name="p1w", bufs=1) as p1w, \
                    tc.tile_pool(name="p1x", bufs=2) as p1x, \
                    tc.tile_pool(name="p1t", bufs=1) as p1t, \
                    tc.tile_pool(name="p1ps", bufs=3, space="PSUM") as p1ps:
                w = p1w.tile([128, NE, DO, 128], BF16)
                # weights issued et-major so the PE's (sb0, et) groups are
                # fed in exactly the order they're consumed
                wv_ = w_t[:].rearrange("p et (do c) -> p et do c", do=DO)
                for dh in range(2):
                    dsl = slice(dh * (DO // 2), (dh + 1) * (DO // 2))
                    nc.scalar.dma_start(w[:, 0, dsl, :], wv_[:, 0, dsl, :])
                for et in range(1, NE):
                    nc.scalar.dma_start(
                        w[:, et].rearrange("p do c -> p (do c)"), w_t[:, et])
                # x for sb0/sb1 prefetched in pieces (first pieces smallest so
                # the PE's first accumulation group starts ASAP)
                xtiles, ctiles, stiles = [], [], []
                for sb in range(2):
                    ssl = slice(sb * XSB, (sb + 1) * XSB)
                    xtile = p1x.tile([128, DO, XSB], BF16, tag="x")
                    chunks = (8, 8, 8, 8) if sb == 0 else (16, 16)
                    d0 = 0
                    for nd in chunks:
                        dsl = slice(d0, d0 + nd)
                        nc.sync.dma_start(
                            xtile[:, dsl, :], xt_t[:, sb, dsl, :])
                        d0 += nd
                    cct = p1x.tile([128, XSB], F32, tag="cc")
                    sst = p1x.tile([128, XSB], F32, tag="ss")
                    nc.sync.dma_start(cct, cc[:][:, ssl])
                    nc.sync.dma_start(sst, ss[:][:, ssl])
                    xtiles.append(xtile)
                    ctiles.append(cct)
                    stiles.append(sst)
                for sb in range(NXSB):
                    ssl = slice(sb * XSB, (sb + 1) * XSB)
                    if sb < 2:
                        xtile, cct, sst = xtiles[sb], ctiles[sb], stiles[sb]
                    else:
                        xtile = p1x.tile([128, DO, XSB], BF16, tag="x")
                        nc.sync.dma_start(xtile, xt_t[:, sb])
                        cct = p1x.tile([128, XSB], F32, tag="cc")
                        sst = p1x.tile([128, XSB], F32, tag="ss")
                        nc.sync.dma_start(cct, cc[:][:, ssl])
                        nc.sync.dma_start(sst, ss[:][:, ssl])
                    atile = p1t.tile([128, HPC + 1, XSB], F32, tag="at")
                    btile = p1t.tile([128, HPC + 1, XSB], F32, tag="bt")
                    bsw = p1t.tile([128, (HPC + 1) * XSB], F32, tag="bsw")
                    bt2 = btile.rearrange("p e s -> p (e s)")
                    bs3 = bsw.rearrange("p (e s) -> p e s", s=XSB)
                    last = sb == NXSB - 1
                    # in the last sub-block, compute k first and finish its
                    # RoPE early so attention's first scores aren't gated on
                    # the whole RoPE tail
                    ets = (4, 0, 1, 2, 3, 5) if last else (0, 1, 2, 3, 4, 5)
                    for et in ets:
                        ps = p1ps.tile([128, XSB], F32, tag="ps")
                        for do in range(DO):
                            nc.tensor.matmul(
                                ps,
                                w[:, et, do, :],
                                xtile[:, do, :],
                                start=(do == 0),
                                stop=(do == DO - 1),
                            )
                        if et < HPC + 1:
                            # t*cos and t*sin halves; swap+add finishes RoPE
                            nc.vector.tensor_mul(atile[:, et, :], ps, cct)
                            nc.vector.tensor_mul(btile[:, et, :], ps, sst)
                            if last:
                                # per-et swap+add so each qe slice is ready
                                # ~2us after its matmuls, shrinking the
                                # phase-1 -> attention bubble
                                esl_ = slice(et * XSB, (et + 1) * XSB)
                                nc.gpsimd.dma_start(
                                    bsw[:64, esl_], bt2[64:, esl_])
                                nc.gpsimd.dma_start(
                                    bsw[64:, esl_], bt2[:64, esl_])
                                nc.vector.tensor_add(
                                    qe[et][:, ssl], atile[:, et, :],
                                    bs3[:, et, :])
                        else:
                            nc.scalar.copy(qe[et][:, ssl], ps)
                    if not last:
                        # half-swap of btile on the (idle) gpsimd DMA queue
                        nc.gpsimd.dma_start(bsw[:64, :], bt2[64:, :])
                        nc.gpsimd.dma_start(bsw[64:, :], bt2[:64, :])
                        for et in range(HPC + 1):
                            nc.vector.tensor_add(
                                qe[et][:, ssl], atile[:, et, :],
                                bs3[:, et, :])
                    # transpose this sb's four v tiles into [t, hd] layout
                    for i in range(4):
                        tt = 4 * sb + i
                        pst = p1ps.tile([128, 128], BF16, tag="tr", bufs=2)
                        nc.tensor.transpose(
                            pst, qe[HPC + 1][:, tt * 128:(tt + 1) * 128], idt)
                        nc.scalar.copy(vn[:, tt, :], pst)

            # ------- Phase 2+3: attention blocks + interleaved out-proj -----
            with tc.tile_pool(name="p3w", bufs=1) as p3w, \
                    tc.tile_pool(name="p2c", bufs=1) as p2c, \
                    tc.tile_pool(name="p2e", bufs=8) as p2e, \
                    tc.tile_pool(name="p2a", bufs=2) as p2a, \
                    tc.tile_pool(name="p2t", bufs=2) as p2t, \
                    tc.tile_pool(name="p3o", bufs=2) as p3o, \
                    tc.tile_pool(name="psS", bufs=2, space="PSUM") as psS, \
                    tc.tile_pool(name="pav", bufs=2, space="PSUM") as pavp, \
                    tc.tile_pool(name="pop", bufs=2, space="PSUM") as popp:
                mt = p2c.tile([128, 2, 2 * SB], BF16)
                nc.sync.dma_start(mt, masks2[:].rearrange("m p s -> p m s"))
                w3 = p3w.tile([128, HPC, D], BF16)
                nc.sync.dma_start(
                    w3, wot[:].rearrange("(eo p) d -> p eo d", p=128))
                ao = [p2c.tile([128, S], BF16, name=f"ao{h}", tag=f"ao{h}")
                      for h in range(HPC)]

                FIRED = {"tiles": set(), "fired": set()}

                def _flush(item):
                    fh, fbsl, fpau, fnrmb = item
                    nc.vector.reciprocal_approx_fast(fnrmb, fnrmb)
                    nc.vector.tensor_mul(ao[fh][:, fbsl], fpau, fnrmb)

                def emit_attn(b):
                    bsl = slice(b * SB, (b + 1) * SB)
                    npair = 2 * b + 2
                    porder = list(range(npair))
                    deferred = []
                    for h in range(HPC):
                        # renorm of head h-2 runs now: its partition reduce is
                        # long done, so the DVE never head-of-line blocks
                        if len(deferred) >= 2:
                            _flush(deferred.pop(0))
                        pa = pavp.tile([128, SB], F32, tag="pav")
                        partials = []
                        for pi, p in enumerate(porder):
                            j0, j1 = 2 * p, 2 * p + 1
                            # diagonal tiles: columns < 128*m are masked for
                            # every partition, so narrow the matmuls to the
                            # live range. exp/mask read the stale (bounded)
                            # PSUM there and the mask zeroes it.
                            lo0 = max(0, j0 - 4 * b) * 128
                            lo1 = max(0, j1 - 4 * b) * 128
                            psc = psS.tile([128, 2 * SB], F32, tag="sc")
                            nc.tensor.matmul(
                                psc[:, lo0:SB],
                                qe[HPC][:, j0 * 128:(j0 + 1) * 128],
                                qe[h][:, b * SB + lo0:(b + 1) * SB],
                                start=True, stop=True)
                            nc.tensor.matmul(
                                psc[:, SB + lo1:],
                                qe[HPC][:, j1 * 128:(j1 + 1) * 128],
                                qe[h][:, b * SB + lo1:(b + 1) * SB],
                                start=True, stop=True)
                            ex = p2e.tile([128, 2 * SB], BF16, tag="ex")
                            nc.scalar.activation(
                                ex, psc, mybir.ActivationFunctionType.Exp)
                            if p >= 2 * b:
                                nc.vector.tensor_mul(
                                    ex, ex, mt[:, p - 2 * b, :])
                            # bf16 pair-sum feeding the softmax-denominator
                            # tree (2x DVE rate; depth<=4 so rounding is tiny)
                            tp = p2t.tile([128, SB], BF16, tag="tp", bufs=12)
                            nc.vector.tensor_add(tp, ex[:, :SB], ex[:, SB:])
                            partials.append(tp)
                            nc.tensor.matmul(
                                pa[:, lo0:], vn[:, j0, :], ex[:, lo0:SB],
                                start=(pi == 0), stop=False)
                            nc.tensor.matmul(
                                pa[:, lo1:], vn[:, j1, :], ex[:, SB + lo1:],
                                start=False, stop=(pi == npair - 1))
                        while len(partials) > 2:
                            nxt = []
                            for k in range(0, len(partials) - 1, 2):
                                u = p2t.tile([128, SB], BF16, tag="tp",
                                             bufs=12)
                                nc.vector.tensor_add(
                                    u, partials[k], partials[k + 1])
                                nxt.append(u)
                            if len(partials) % 2:
                                nxt.append(partials[-1])
                            partials = nxt
                        acc = p2a.tile([128, SB], F32, tag="acc")
                        nc.vector.tensor_add(acc, partials[0], partials[1])
                        nrmb = p2a.tile([128, SB], F32, tag="nrm", bufs=4)
                        nc.gpsimd.partition_all_reduce(
                            nrmb, acc, 128, bass_isa.ReduceOp.add)
                        # evacuate pa so its PSUM bank recycles immediately
                        pau = p2a.tile([128, SB], F32, tag="pau", bufs=4)
                        nc.vector.tensor_copy(pau, pa)
                        deferred.append((h, bsl, pau, nrmb))
                    for item in deferred:
                        _flush(item)

                def emit_oproj(b):
                    # out-projection for this block's four s-tiles
                    done_tiles = FIRED["tiles"]
                    for stl_i in range(4):
                        st = b * 4 + stl_i
                        st0 = st * 128
                        ci = next(i for i, (t0, t1) in enumerate(CH)
                                  if t0 <= st < t1)
                        t0, t1 = CH[ci]
                        riv = rs_in[ci].rearrange("(t p) d -> p t d", p=128)
                        osb = p3o.tile([128, D], BF16, tag="osb")
                        for db in range(D // SB):
                            po = popp.tile([128, SB], F32, tag="pop")
                            for hh in range(HPC):
                                nc.tensor.matmul(
                                    po,
                                    ao[hh][:, st0:st0 + 128],
                                    w3[:, hh, db * SB:(db + 1) * SB],
                                    start=(hh == 0), stop=(hh == HPC - 1))
                            nc.scalar.copy(
                                osb[:, db * SB:(db + 1) * SB], po)
                        nc.sync.dma_start(riv[:, st - t0, :], osb)
                        done_tiles.add(st)
                        for ci2, (u0, u1) in enumerate(CH):
                            if ci2 not in FIRED["fired"] and all(
                                    t in done_tiles for t in range(u0, u1)):
                                FIRED["fired"].add(ci2)
                                nc.gpsimd.collective_compute(
                                    "ReduceScatter",
                                    mybir.AluOpType.add,
                                    ins=[rs_in[ci2].opt()],
                                    outs=[rs_out[ci2].opt()],
                                    replica_groups=RG,
                                )

                # Virtual ready-times pin the static schedule to this phase
                # order — without them the scheduler hoists out-proj
                # LDWEIGHTS/ship DMAs into earlier phases where their guards
                # stall the whole engine stream. Out-proj directly follows
                # each block so the ReduceScatter pipeline starts as early
                # as possible (it is the second-half critical path).
                emit_attn(3)
                with tc.tile_wait_until(1):
                    emit_oproj(3)
                with tc.tile_wait_until(2):
                    emit_attn(2)
                with tc.tile_wait_until(3):
                    emit_oproj(2)
                with tc.tile_wait_until(4):
                    emit_attn(1)
                with tc.tile_wait_until(5):
                    emit_oproj(1)
                with tc.tile_wait_until(6):
                    emit_attn(0)
                with tc.tile_wait_until(7):
                    emit_oproj(0)
                # tail: ship RS shards (bf16) straight to the output; host
                # upcasts to fp32
                # ship on the scalar HWDGE queue: the gpsimd queue is strict
                # FIFO and a ship DMA there would block the partition
                # reduces queued behind it for a whole collective
                orow = 0
                with tc.tile_wait_until(8):
                    for ci, (t0, t1) in enumerate(CH):
                        nr = (t1 - t0) * 16
                        nc.sync.dma_start(
                            out[:][orow:orow + nr, :], rs_out[ci][:])
                        orow += nr
    nc.compile()
    return nc


_CACHE = {}


def _get_program():
    if "nc" not in _CACHE:
        _CACHE["nc"] = build()
    return _CACHE["nc"]


def _host_prep(x, freqs_cos, freqs_sin, wq, wk, wv, wo):
    x2 = np.ascontiguousarray(np.asarray(x, np.float32).reshape(S, D))
    # partition-major repack: [p, sb, do, c] = xT[do*128+p, sb*512+c]
    xT = np.ascontiguousarray(
        x2.T.reshape(DO, 128, NXSB, XSB).transpose(1, 2, 0, 3)
        .reshape(128, NXSB * DO * XSB)).astype(NPBF)
    # even|odd -> [evens;odds] row permutation per head (RoPE partition split)
    perm1 = np.concatenate([np.arange(0, HD, 2), np.arange(1, HD, 2)])
    permq = (np.arange(H)[:, None] * HD + perm1[None, :]).reshape(-1)
    permk = (np.arange(KV)[:, None] * HD + perm1[None, :]).reshape(-1)
    scale = np.float32(1.0 / np.sqrt(HD))
    wq_p = np.asarray(wq, np.float32)[permq] * scale
    wk_p = np.asarray(wk, np.float32)[permk]
    wv32 = np.asarray(wv, np.float32)
    wo32 = np.asarray(wo, np.float32)
    cosT = np.asarray(freqs_cos, np.float32).T
    sinT = np.asarray(freqs_sin, np.float32).T
    ccb = np.ascontiguousarray(np.concatenate([cosT, cosT], 0))
    ssb = np.ascontiguousarray(np.concatenate([sinT, -sinT], 0))
    tp = np.arange(128, dtype=np.int64)[:, None]
    sf = np.arange(SB, dtype=np.int64)[None, :]
    masks = [(sf >= tp + 128 * m).astype(NPBF) for m in range(HPC)]
    masks2 = np.stack([np.concatenate([masks[0], masks[1]], 1),
                       np.concatenate([masks[2], masks[3]], 1)], 0)
    ident = np.eye(128, dtype=NPBF)

    in_maps = []
    for i in range(NCORES):
        wqkv = np.concatenate(
            [wq_p[i * EQ:(i + 1) * EQ],
             wk_p[i * HD:(i + 1) * HD],
             wv32[i * HD:(i + 1) * HD]], 0)  # [768, 4096]
        # partition-major repack: [p, et, do, c] with
        # element = wqkv[et*128 + c, do*128 + p]
        wqkvt = np.ascontiguousarray(
            wqkv.reshape(NE, 128, DO, 128).transpose(3, 0, 2, 1)
            .reshape(128, NE * DO * 128)).astype(NPBF)
        wot = np.ascontiguousarray(
            wo32[:, i * EQ:(i + 1) * EQ].T).astype(NPBF)  # [512, 4096]
        in_maps.append(dict(xt=xT, wqkvt=wqkvt, wot=wot, cc=ccb, ss=ssb,
                            masks2=masks2, ident=ident))
    return in_maps


def _run(in_maps, trace=False):
    nc = _get_program()
    return run_bass_kernel_spmd(
        nc, in_maps, core_ids=list(range(NCORES)), trace=trace)


CH_HOST = list(CH)


def _assemble(res):
    full = np.empty((S, D), np.float32)
    for r in range(NCORES):
        shard = np.asarray(res.results[r]["out"]).astype(np.float32)
        orow = 0
        for (t0, t1) in CH_HOST:
            nr = (t1 - t0) * 16
            full[t0 * 128 + r * nr: t0 * 128 + (r + 1) * nr, :] = \
                shard[orow:orow + nr, :]
            orow += nr
    return full.reshape(B, S, D)


def kernel(x, freqs_cos, freqs_sin, wq, wk, wv, wo):
    in_maps = _host_prep(x, freqs_cos, freqs_sin, wq, wk, wv, wo)
    res = _run(in_maps, trace=False)
    return _assemble(res)


def _build_sharded():
    """Mirror of bass2jax.run_bass_via_pjrt's multi-core path, split so the
    jitted callable and device-resident inputs can be reused for timing."""
    import jax
    from jax.experimental.shard_map import shard_map
    from jax.sharding import Mesh, PartitionSpec

    import concourse.mybir as mb
    from concourse import bass2jax

    nc = _get_program()
    bass2jax.install_neuronx_cc_hook()
    part_name = (nc.partition_id_tensor.name
                 if nc.partition_id_tensor else None)
    in_names, out_names, out_avals, zero_outs = [], [], [], []
    for alloc in nc.m.functions[0].allocations:
        if not isinstance(alloc, mb.MemoryLocationSet):
            continue
        name = alloc.memorylocations[0].name
        if alloc.kind == "ExternalInput":
            if name != part_name:
                in_names.append(name)
        elif alloc.kind == "ExternalOutput":
            out_names.append(name)
            shape = tuple(alloc.tensor_shape)
            dtype = mb.dt.np(alloc.dtype)
            out_avals.append(jax.core.ShapedArray(shape, dtype))
            zero_outs.append(np.zeros(shape, dtype))
    n_params = len(in_names)
    all_names = in_names + out_names
    if part_name is not None:
        all_names = all_names + [part_name]

    def _body(*args):
        operands = list(args)
        if part_name is not None:
            operands.append(bass2jax.partition_id_tensor())
        outs = bass2jax._bass_exec_p.bind(
            *operands,
            out_avals=tuple(out_avals),
            in_names=tuple(all_names),
            out_names=tuple(out_names),
            lowering_input_output_aliases=(),
            sim_require_finite=True,
            sim_require_nnan=True,
            nc=nc,
        )
        return tuple(outs)

    devices = jax.devices()[:NCORES]
    mesh = Mesh(np.asarray(devices), ("core",))
    n_outs = len(out_names)
    sharded = jax.jit(
        shard_map(
            _body, mesh=mesh,
            in_specs=(PartitionSpec("core"),) * (n_params + n_outs),
            out_specs=(PartitionSpec("core"),) * n_outs,
            check_rep=False,
        ),
        donate_argnums=tuple(range(n_params, n_params + n_outs)),
        keep_unused=True,
    )
    return sharded, in_names, out_names, out_avals, zero_outs, mesh


def kernel_profiled(x, freqs_cos, freqs_sin, wq, wk, wv, wo, iters=12):
    """Returns (output, per-execution wall ns). Times repeated on-device
    executions with inputs pre-placed on the devices."""
    import time

    import jax
    from jax.sharding import NamedSharding, PartitionSpec

    in_maps = _host_prep(x, freqs_cos, freqs_sin, wq, wk, wv, wo)
    sharded, in_names, out_names, out_avals, zero_outs, mesh = _build_sharded()
    spec = NamedSharding(mesh, PartitionSpec("core"))
    concat_in = [
        jax.device_put(
            np.concatenate([in_maps[c][n] for c in range(NCORES)], axis=0),
            spec)
        for n in in_names
    ]

    def zeros():
        return [
            jax.device_put(
                np.zeros((NCORES * z.shape[0], *z.shape[1:]), z.dtype), spec)
            for z in zero_outs
        ]

    out_arrs = sharded(*concat_in, *zeros())  # warmup & result
    jax.block_until_ready(out_arrs)
    result = [np.asarray(a) for a in out_arrs]

    zsets = [zeros() for _ in range(iters)]
    jax.block_until_ready(zsets)
    t0 = time.perf_counter()
    last = None
    for zs in zsets:
        last = sharded(*concat_in, *zs)
    jax.block_until_ready(last)
    t1 = time.perf_counter()
    per_iter_ns = (t1 - t0) / iters * 1e9

    res_maps = [
        {n: result[i].reshape(NCORES, *out_avals[i].shape)[c]
         for i, n in enumerate(out_names)}
        for c in range(NCORES)
    ]

    class _R:
        results = res_maps

    return _assemble(_R), per_iter_ns


def _enable_ntff_hook():
    """Synthesize antenv.axon_hooks (absent in this image) and register the
    ctypes NTFF profile hook so run_bass_kernel_spmd(trace=True) works."""
    import sys as _sys
    import types as _types

    if "antenv.axon_hooks" in _sys.modules:
        return
    import antenv  # noqa: F401
    mod = _types.ModuleType("antenv.axon_hooks")
    mod._hook = None

    def set_axon_ntff_profile_hook(h):
        mod._hook = h

    def get_axon_ntff_profile_hook():
        return mod._hook

    mod.set_axon_ntff_profile_hook = set_axon_ntff_profile_hook
    mod.get_axon_ntff_profile_hook = get_axon_ntff_profile_hook
    _sys.modules["antenv.axon_hooks"] = mod
    antenv.axon_hooks = mod
    from trn_agent_boot.trn_boot import _ntff_profile_via_ctypes
    hook = _ntff_profile_via_ctypes("/opt/axon/libaxon_pjrt.so")
    if hook is not None:
        mod.set_axon_ntff_profile_hook(hook)
    # uploads need a fish bucket this container lacks; neuter them
    import concourse.bass_utils as _bu
    _bu.upload_artifacts = lambda tmpdir: f"local:{tmpdir}"


def kernel_traced(x, freqs_cos, freqs_sin, wq, wk, wv, wo, tmpdir=None):
    """Run once with NTFF tracing; returns (output, BassKernelResults)."""
    _enable_ntff_hook()
    in_maps = _host_prep(x, freqs_cos, freqs_sin, wq, wk, wv, wo)
    nc = _get_program()
    res = run_bass_kernel_spmd(
        nc, in_maps, core_ids=list(range(NCORES)), trace=True, tmpdir=tmpdir)
    return _assemble(res), res


# revision 60
# speedup vs baseline: 1.0946x; 1.0424x over previous
"""GQA causal attention (B=1, S=2048, D=4096, H=32, KV=8) on 8 trn2 cores.

Strategy: tensor-parallel over heads. Core i owns q-heads 4i..4i+3 and
kv-head i. Host pre-transposes weights/x so every matmul contracts along
the partition dim, and pre-permutes wq/wk rows (even|odd interleave ->
[evens;odds]) so RoPE becomes partition-aligned elementwise math.
Attention is computed head-locally in a scores^T [t, s] layout; softmax
sums are accumulated on the vector engine and reduced across partitions
with one gpsimd partition_all_reduce per (head, block) — no norm matmul
stream on the PE. Exp runs on 1024-wide pairs of score tiles to amortize
the activation engine's per-instruction overhead. After each 512-row
block of attention, the local out-projection shard is computed and a
bf16 ReduceScatter is fired as soon as each row-chunk completes, so the
collectives overlap compute. Host concatenates the per-core shards.

Matmul operands are bf16; accumulation, softmax and RoPE math are fp32.
"""

import sys

import numpy as np

sys.path.insert(0, "/opt/trn_rl_repo")

import ml_dtypes  # noqa: E402

import concourse.bass as bass  # noqa: E402
from concourse import bacc  # noqa: E402
from concourse import bass_isa  # noqa: E402
import concourse.mybir as mybir  # noqa: E402
import concourse.tile as tile  # noqa: E402
from concourse.bass_utils import run_bass_kernel_spmd  # noqa: E402

F32 = mybir.dt.float32
BF16 = mybir.dt.bfloat16
NPBF = ml_dtypes.bfloat16

B, S, D = 1, 2048, 4096
H, KV, HD = 32, 8, 128
NCORES = 8
HPC = H // NCORES  # q heads per core = 4
EQ = HPC * HD  # 512 local q features
NE = HPC + 2  # e-tiles per core: 4 q + 1 k + 1 v
SB = 512  # attention s block
NSB = S // SB  # 4
XSB = 512  # phase-1 s sub-block (matmul moving dim)
NXSB = S // XSB  # 4
DO = D // 128  # 32 contraction tiles for projections
TT = S // 128  # 16 t-tiles
RG = [list(range(NCORES))]
CH = [(12, 14), (14, 16), (8, 12), (4, 8), (0, 2), (2, 4)]


def build():
    nc = bacc.Bacc("TRN2", target_bir_lowering=False)
    # partition-major: [p, sb, do, c] so each phase-1 piece is a 16KB
    # contiguous run per partition (few, large DMA descriptors)
    xt = nc.dram_tensor("xt", [128, NXSB * DO * XSB], BF16,
                        kind="ExternalInput")
    # partition-major layout: [p, et, do, c] so each per-et DMA moves 8KB
    # contiguous per partition (line-rate) in PE consumption order
    wqkvt = nc.dram_tensor("wqkvt", [128, NE * DO * 128], BF16,
                           kind="ExternalInput")
    wot = nc.dram_tensor("wot", [EQ, D], BF16, kind="ExternalInput")
    cc = nc.dram_tensor("cc", [128, S], F32, kind="ExternalInput")
    ss = nc.dram_tensor("ss", [128, S], F32, kind="ExternalInput")
    masks2 = nc.dram_tensor("masks2", [2, 128, 2 * SB], BF16,
                            kind="ExternalInput")
    ident = nc.dram_tensor("ident", [128, 128], BF16, kind="ExternalInput")
    ones = nc.dram_tensor("ones", [128, 1], BF16, kind="ExternalInput")
    out = nc.dram_tensor("out", [NSB * 64, D], BF16, kind="ExternalOutput")

    xt_t = xt[:].rearrange("p (sb do c) -> p sb do c", sb=NXSB, do=DO)
    w_t = wqkvt[:].rearrange("p (et do c) -> p et (do c)", et=NE, do=DO)

    with tile.TileContext(nc) as tc:
        with tc.tile_pool(name="dram", bufs=1, space="DRAM") as dram, \
                tc.tile_pool(name="pqkv", bufs=1) as pqkv:
            rs_in = [dram.tile([(t1 - t0) * 128, D], BF16, name=f"rsi{ci}")
                     for ci, (t0, t1) in enumerate(CH)]
            rs_out = [dram.tile([(t1 - t0) * 16, D], BF16, name=f"rso{ci}")
                      for ci, (t0, t1) in enumerate(CH)]
            qe = [pqkv.tile([128, S], BF16, name=f"qe{et}", tag=f"qe{et}")
                  for et in range(NE)]
            # v in natural [t, hd] tiles (filled by per-sb PE transposes)
            vn = pqkv.tile([128, TT, HD], BF16, tag="vn")
            idt = pqkv.tile([128, 128], BF16, tag="idt")
            nc.scalar.dma_start(idt, ident[:])

            # ---------------- Phase 1: fused QKV projection + RoPE ----------
            with tc.tile_pool(name="p1w", bufs=1) as p1w, \
                    tc.tile_pool(name="p1x", bufs=2) as p1x, \
                    tc.tile_pool(name="p1t", bufs=1) as p1t, \
                    tc.tile_pool(name="p1ps", bufs=3, space="PSUM") as p1ps:
                w = p1w.tile([128, NE, DO, 128], BF16)
                # weights issued et-major so the PE's (sb0, et) groups are
                # fed in exactly the order they're consumed
                wv_ = w_t[:].rearrange("p et (do c) -> p et do c", do=DO)
                for dh in range(2):
                    dsl = slice(dh * (DO // 2), (dh + 1) * (DO // 2))
                    nc.scalar.dma_start(w[:, 0, dsl, :], wv_[:, 0, dsl, :])
                for et in range(1, NE):
                    nc.scalar.dma_start(
                        w[:, et].rearrange("p do c -> p (do c)"), w_t[:, et])
                # x for sb0/sb1 prefetched in pieces (first pieces smallest so
                # the PE's first accumulation group starts ASAP)
                xtiles, ctiles, stiles = [], [], []
                for sb in range(2):
                    ssl = slice(sb * XSB, (sb + 1) * XSB)
                    xtile = p1x.tile([128, DO, XSB], BF16, tag="x")
                    chunks = (8, 8, 8, 8) if sb == 0 else (16, 16)
                    d0 = 0
                    for nd in chunks:
                        dsl = slice(d0, d0 + nd)
                        nc.sync.dma_start(
                            xtile[:, dsl, :], xt_t[:, sb, dsl, :])
                        d0 += nd
                    cct = p1x.tile([128, XSB], F32, tag="cc")
                    sst = p1x.tile([128, XSB], F32, tag="ss")
                    nc.sync.dma_start(cct, cc[:][:, ssl])
                    nc.sync.dma_start(sst, ss[:][:, ssl])
                    xtiles.append(xtile)
                    ctiles.append(cct)
                    stiles.append(sst)
                for sb in range(NXSB):
                    ssl = slice(sb * XSB, (sb + 1) * XSB)
                    if sb < 2:
                        xtile, cct, sst = xtiles[sb], ctiles[sb], stiles[sb]
                    else:
                        xtile = p1x.tile([128, DO, XSB], BF16, tag="x")
                        nc.sync.dma_start(xtile, xt_t[:, sb])
                        cct = p1x.tile([128, XSB], F32, tag="cc")
                        sst = p1x.tile([128, XSB], F32, tag="ss")
                        nc.sync.dma_start(cct, cc[:][:, ssl])
                        nc.sync.dma_start(sst, ss[:][:, ssl])
                    atile = p1t.tile([128, HPC + 1, XSB], F32, tag="at")
                    btile = p1t.tile([128, HPC + 1, XSB], F32, tag="bt")
                    bsw = p1t.tile([128, (HPC + 1) * XSB], F32, tag="bsw")
                    bt2 = btile.rearrange("p e s -> p (e s)")
                    bs3 = bsw.rearrange("p (e s) -> p e s", s=XSB)
                    last = sb == NXSB - 1
                    # in the last sub-block, compute k first and finish its
                    # RoPE early so attention's first scores aren't gated on
                    # the whole RoPE tail
                    ets = (4, 0, 1, 2, 3, 5) if last else (0, 1, 2, 3, 4, 5)
                    for et in ets:
                        ps = p1ps.tile([128, XSB], F32, tag="ps")
                        for do in range(DO):
                            nc.tensor.matmul(
                                ps,
                                w[:, et, do, :],
                                xtile[:, do, :],
                                start=(do == 0),
                                stop=(do == DO - 1),
                            )
                        if et < HPC + 1:
                            # t*cos and t*sin halves; swap+add finishes RoPE
                            nc.vector.tensor_mul(atile[:, et, :], ps, cct)
                            nc.vector.tensor_mul(btile[:, et, :], ps, sst)
                            if last:
                                # per-et swap+add so each qe slice is ready
                                # ~2us after its matmuls, shrinking the
                                # phase-1 -> attention bubble
                                esl_ = slice(et * XSB, (et + 1) * XSB)
                                nc.gpsimd.dma_start(
                                    bsw[:64, esl_], bt2[64:, esl_])
                                nc.gpsimd.dma_start(
                                    bsw[64:, esl_], bt2[:64, esl_])
                                nc.vector.tensor_add(
                                    qe[et][:, ssl], atile[:, et, :],
                                    bs3[:, et, :])
                        else:
                            nc.scalar.copy(qe[et][:, ssl], ps)
                    if not last:
                        # half-swap of btile on the (idle) gpsimd DMA queue
                        nc.gpsimd.dma_start(bsw[:64, :], bt2[64:, :])
                        nc.gpsimd.dma_start(bsw[64:, :], bt2[:64, :])
                        for et in range(HPC + 1):
                            nc.vector.tensor_add(
                                qe[et][:, ssl], atile[:, et, :],
                                bs3[:, et, :])
                    # transpose this sb's four v tiles into [t, hd] layout
                    for i in range(4):
                        tt = 4 * sb + i
                        pst = p1ps.tile([128, 128], BF16, tag="tr", bufs=2)
                        nc.tensor.transpose(
                            pst, qe[HPC + 1][:, tt * 128:(tt + 1) * 128], idt)
                        nc.scalar.copy(vn[:, tt, :], pst)

            # ------- Phase 2+3: attention blocks + interleaved out-proj -----
            with tc.tile_pool(name="p3w", bufs=1) as p3w, \
                    tc.tile_pool(name="p2c", bufs=1) as p2c, \
                    tc.tile_pool(name="p2e", bufs=8) as p2e, \
                    tc.tile_pool(name="p2a", bufs=2) as p2a, \
                    tc.tile_pool(name="p2t", bufs=2) as p2t, \
                    tc.tile_pool(name="p3o", bufs=2) as p3o, \
                    tc.tile_pool(name="psS", bufs=2, space="PSUM") as psS, \
                    tc.tile_pool(name="pav", bufs=2, space="PSUM") as pavp, \
                    tc.tile_pool(name="pop", bufs=2, space="PSUM") as popp:
                mt = p2c.tile([128, 2, 2 * SB], BF16)
                nc.sync.dma_start(mt, masks2[:].rearrange("m p s -> p m s"))
                on = p2c.tile([128, 1], BF16)
                nc.sync.dma_start(on, ones[:])
                w3 = p3w.tile([128, HPC, D], BF16)
                nc.sync.dma_start(
                    w3, wot[:].rearrange("(eo p) d -> p eo d", p=128))
                ao = [p2c.tile([128, S], BF16, name=f"ao{h}", tag=f"ao{h}")
                      for h in range(HPC)]

                FIRED = {"tiles": set(), "fired": set()}

                def _flush(item):
                    fh, fbsl, fpau, fnrmb = item
                    nc.vector.tensor_mul(ao[fh][:, fbsl], fpau, fnrmb)

                def emit_attn(b):
                    bsl = slice(b * SB, (b + 1) * SB)
                    npair = 2 * b + 2
                    porder = list(range(npair))
                    deferred = []
                    for h in range(HPC):
                        # renorm of head h-2 runs now: its partition reduce is
                        # long done, so the DVE never head-of-line blocks
                        if len(deferred) >= 2:
                            _flush(deferred.pop(0))
                        pa = pavp.tile([128, SB], F32, tag="pav")
                        partials = []
                        for pi, p in enumerate(porder):
                            j0, j1 = 2 * p, 2 * p + 1
                            # diagonal tiles: columns < 128*m are masked for
                            # every partition, so narrow the matmuls to the
                            # live range. exp/mask read the stale (bounded)
                            # PSUM there and the mask zeroes it.
                            lo0 = max(0, j0 - 4 * b) * 128
                            lo1 = max(0, j1 - 4 * b) * 128
                            psc = psS.tile([128, 2 * SB], F32, tag="sc")
                            nc.tensor.matmul(
                                psc[:, lo0:SB],
                                qe[HPC][:, j0 * 128:(j0 + 1) * 128],
                                qe[h][:, b * SB + lo0:(b + 1) * SB],
                                start=True, stop=True)
                            nc.tensor.matmul(
                                psc[:, SB + lo1:],
                                qe[HPC][:, j1 * 128:(j1 + 1) * 128],
                                qe[h][:, b * SB + lo1:(b + 1) * SB],
                                start=True, stop=True)
                            ex = p2e.tile([128, 2 * SB], BF16, tag="ex")
                            nc.scalar.activation(
                                ex, psc, mybir.ActivationFunctionType.Exp)
                            if p >= 2 * b:
                                nc.vector.tensor_mul(
                                    ex, ex, mt[:, p - 2 * b, :])
                            # bf16 pair-sum feeding the softmax-denominator
                            # tree (2x DVE rate; depth<=4 so rounding is tiny)
                            tp = p2t.tile([128, SB], BF16, tag="tp", bufs=12)
                            nc.vector.tensor_add(tp, ex[:, :SB], ex[:, SB:])
                            partials.append(tp)
                            nc.tensor.matmul(
                                pa[:, lo0:], vn[:, j0, :], ex[:, lo0:SB],
                                start=(pi == 0), stop=False)
                            nc.tensor.matmul(
                                pa[:, lo1:], vn[:, j1, :], ex[:, SB + lo1:],
                                start=False, stop=(pi == npair - 1))
                        while len(partials) > 2:
                            nxt = []
                            for k in range(0, len(partials) - 1, 2):
                                u = p2t.tile([128, SB], BF16, tag="tp",
                                             bufs=12)
                                nc.vector.tensor_add(
                                    u, partials[k], partials[k + 1])
                                nxt.append(u)
                            if len(partials) % 2:
                                nxt.append(partials[-1])
                            partials = nxt
                        acc = p2a.tile([128, SB], BF16, tag="acc")
                        nc.vector.tensor_add(acc, partials[0], partials[1])
                        # partition-sum via a ones-matmul into a borrowed
                        # out-proj PSUM slot (idle during attention), then a
                        # cheap broadcast: ~2.5us less chain latency per head
                        # than gpsimd partition_all_reduce, and the gpsimd
                        # queue stays clear for the collective triggers
                        pn = popp.tile([128, SB], F32, tag="pop")
                        nc.tensor.matmul(pn[:1, :], on, acc,
                                         start=True, stop=True)
                        nrm1 = p2a.tile([1, SB], F32, tag="nrm1", bufs=4)
                        nc.vector.reciprocal_approx_fast(nrm1, pn[:1, :])
                        nrmb = p2a.tile([128, SB], F32, tag="nrm", bufs=4)
                        nc.gpsimd.partition_broadcast(nrmb, nrm1)
                        # evacuate pa so its PSUM bank recycles immediately
                        pau = p2a.tile([128, SB], F32, tag="pau", bufs=4)
                        nc.vector.tensor_copy(pau, pa)
                        deferred.append((h, bsl, pau, nrmb))
                    for item in deferred:
                        _flush(item)

                def emit_oproj(b):
                    # out-projection for this block's four s-tiles
                    done_tiles = FIRED["tiles"]
                    for stl_i in range(4):
                        st = b * 4 + stl_i
                        st0 = st * 128
                        ci = next(i for i, (t0, t1) in enumerate(CH)
                                  if t0 <= st < t1)
                        t0, t1 = CH[ci]
                        riv = rs_in[ci].rearrange("(t p) d -> p t d", p=128)
                        osb = p3o.tile([128, D], BF16, tag="osb")
                        for db in range(D // SB):
                            po = popp.tile([128, SB], F32, tag="pop")
                            for hh in range(HPC):
                                nc.tensor.matmul(
                                    po,
                                    ao[hh][:, st0:st0 + 128],
                                    w3[:, hh, db * SB:(db + 1) * SB],
                                    start=(hh == 0), stop=(hh == HPC - 1))
                            nc.scalar.copy(
                                osb[:, db * SB:(db + 1) * SB], po)
                        nc.sync.dma_start(riv[:, st - t0, :], osb)
                        done_tiles.add(st)
                        for ci2, (u0, u1) in enumerate(CH):
                            if ci2 not in FIRED["fired"] and all(
                                    t in done_tiles for t in range(u0, u1)):
                                FIRED["fired"].add(ci2)
                                nc.gpsimd.collective_compute(
                                    "ReduceScatter",
                                    mybir.AluOpType.add,
                                    ins=[rs_in[ci2].opt()],
                                    outs=[rs_out[ci2].opt()],
                                    replica_groups=RG,
                                )

                # Virtual ready-times pin the static schedule to this phase
                # order — without them the scheduler hoists out-proj
                # LDWEIGHTS/ship DMAs into earlier phases where their guards
                # stall the whole engine stream. Out-proj directly follows
                # each block so the ReduceScatter pipeline starts as early
                # as possible (it is the second-half critical path).
                emit_attn(3)
                with tc.tile_wait_until(1):
                    emit_oproj(3)
                with tc.tile_wait_until(2):
                    emit_attn(2)
                with tc.tile_wait_until(3):
                    emit_oproj(2)
                with tc.tile_wait_until(4):
                    emit_attn(1)
                with tc.tile_wait_until(5):
                    emit_oproj(1)
                with tc.tile_wait_until(6):
                    emit_attn(0)
                with tc.tile_wait_until(7):
                    emit_oproj(0)
                # tail: ship RS shards (bf16) straight to the output; host
                # upcasts to fp32
                # ship on the scalar HWDGE queue: the gpsimd queue is strict
                # FIFO and a ship DMA there would block the partition
                # reduces queued behind it for a whole collective
                orow = 0
                with tc.tile_wait_until(8):
                    for ci, (t0, t1) in enumerate(CH):
                        nr = (t1 - t0) * 16
                        nc.sync.dma_start(
                            out[:][orow:orow + nr, :], rs_out[ci][:])
                        orow += nr
    nc.compile()
    return nc


_CACHE = {}


def _get_program():
    if "nc" not in _CACHE:
        _CACHE["nc"] = build()
    return _CACHE["nc"]


def _host_prep(x, freqs_cos, freqs_sin, wq, wk, wv, wo):
    x2 = np.ascontiguousarray(np.asarray(x, np.float32).reshape(S, D))
    # partition-major repack: [p, sb, do, c] = xT[do*128+p, sb*512+c]
    xT = np.ascontiguousarray(
        x2.T.reshape(DO, 128, NXSB, XSB).transpose(1, 2, 0, 3)
        .reshape(128, NXSB * DO * XSB)).astype(NPBF)
    # even|odd -> [evens;odds] row permutation per head (RoPE partition split)
    perm1 = np.concatenate([np.arange(0, HD, 2), np.arange(1, HD, 2)])
    permq = (np.arange(H)[:, None] * HD + perm1[None, :]).reshape(-1)
    permk = (np.arange(KV)[:, None] * HD + perm1[None, :]).reshape(-1)
    scale = np.float32(1.0 / np.sqrt(HD))
    wq_p = np.asarray(wq, np.float32)[permq] * scale
    wk_p = np.asarray(wk, np.float32)[permk]
    wv32 = np.asarray(wv, np.float32)
    wo32 = np.asarray(wo, np.float32)
    cosT = np.asarray(freqs_cos, np.float32).T
    sinT = np.asarray(freqs_sin, np.float32).T
    ccb = np.ascontiguousarray(np.concatenate([cosT, cosT], 0))
    ssb = np.ascontiguousarray(np.concatenate([sinT, -sinT], 0))
    tp = np.arange(128, dtype=np.int64)[:, None]
    sf = np.arange(SB, dtype=np.int64)[None, :]
    masks = [(sf >= tp + 128 * m).astype(NPBF) for m in range(HPC)]
    masks2 = np.stack([np.concatenate([masks[0], masks[1]], 1),
                       np.concatenate([masks[2], masks[3]], 1)], 0)
    ident = np.eye(128, dtype=NPBF)
    ones_h = np.ones((128, 1), NPBF)

    in_maps = []
    for i in range(NCORES):
        wqkv = np.concatenate(
            [wq_p[i * EQ:(i + 1) * EQ],
             wk_p[i * HD:(i + 1) * HD],
             wv32[i * HD:(i + 1) * HD]], 0)  # [768, 4096]
        # partition-major repack: [p, et, do, c] with
        # element = wqkv[et*128 + c, do*128 + p]
        wqkvt = np.ascontiguousarray(
            wqkv.reshape(NE, 128, DO, 128).transpose(3, 0, 2, 1)
            .reshape(128, NE * DO * 128)).astype(NPBF)
        wot = np.ascontiguousarray(
            wo32[:, i * EQ:(i + 1) * EQ].T).astype(NPBF)  # [512, 4096]
        in_maps.append(dict(xt=xT, wqkvt=wqkvt, wot=wot, cc=ccb, ss=ssb,
                            masks2=masks2, ident=ident, ones=ones_h))
    return in_maps


def _run(in_maps, trace=False):
    nc = _get_program()
    return run_bass_kernel_spmd(
        nc, in_maps, core_ids=list(range(NCORES)), trace=trace)


CH_HOST = list(CH)


def _assemble(res):
    full = np.empty((S, D), np.float32)
    for r in range(NCORES):
        shard = np.asarray(res.results[r]["out"]).astype(np.float32)
        orow = 0
        for (t0, t1) in CH_HOST:
            nr = (t1 - t0) * 16
            full[t0 * 128 + r * nr: t0 * 128 + (r + 1) * nr, :] = \
                shard[orow:orow + nr, :]
            orow += nr
    return full.reshape(B, S, D)


def kernel(x, freqs_cos, freqs_sin, wq, wk, wv, wo):
    in_maps = _host_prep(x, freqs_cos, freqs_sin, wq, wk, wv, wo)
    res = _run(in_maps, trace=False)
    return _assemble(res)


def _build_sharded():
    """Mirror of bass2jax.run_bass_via_pjrt's multi-core path, split so the
    jitted callable and device-resident inputs can be reused for timing."""
    import jax
    from jax.experimental.shard_map import shard_map
    from jax.sharding import Mesh, PartitionSpec

    import concourse.mybir as mb
    from concourse import bass2jax

    nc = _get_program()
    bass2jax.install_neuronx_cc_hook()
    part_name = (nc.partition_id_tensor.name
                 if nc.partition_id_tensor else None)
    in_names, out_names, out_avals, zero_outs = [], [], [], []
    for alloc in nc.m.functions[0].allocations:
        if not isinstance(alloc, mb.MemoryLocationSet):
            continue
        name = alloc.memorylocations[0].name
        if alloc.kind == "ExternalInput":
            if name != part_name:
                in_names.append(name)
        elif alloc.kind == "ExternalOutput":
            out_names.append(name)
            shape = tuple(alloc.tensor_shape)
            dtype = mb.dt.np(alloc.dtype)
            out_avals.append(jax.core.ShapedArray(shape, dtype))
            zero_outs.append(np.zeros(shape, dtype))
    n_params = len(in_names)
    all_names = in_names + out_names
    if part_name is not None:
        all_names = all_names + [part_name]

    def _body(*args):
        operands = list(args)
        if part_name is not None:
            operands.append(bass2jax.partition_id_tensor())
        outs = bass2jax._bass_exec_p.bind(
            *operands,
            out_avals=tuple(out_avals),
            in_names=tuple(all_names),
            out_names=tuple(out_names),
            lowering_input_output_aliases=(),
            sim_require_finite=True,
            sim_require_nnan=True,
            nc=nc,
        )
        return tuple(outs)

    devices = jax.devices()[:NCORES]
    mesh = Mesh(np.asarray(devices), ("core",))
    n_outs = len(out_names)
    sharded = jax.jit(
        shard_map(
            _body, mesh=mesh,
            in_specs=(PartitionSpec("core"),) * (n_params + n_outs),
            out_specs=(PartitionSpec("core"),) * n_outs,
            check_rep=False,
        ),
        donate_argnums=tuple(range(n_params, n_params + n_outs)),
        keep_unused=True,
    )
    return sharded, in_names, out_names, out_avals, zero_outs, mesh


def kernel_profiled(x, freqs_cos, freqs_sin, wq, wk, wv, wo, iters=12):
    """Returns (output, per-execution wall ns). Times repeated on-device
    executions with inputs pre-placed on the devices."""
    import time

    import jax
    from jax.sharding import NamedSharding, PartitionSpec

    in_maps = _host_prep(x, freqs_cos, freqs_sin, wq, wk, wv, wo)
    sharded, in_names, out_names, out_avals, zero_outs, mesh = _build_sharded()
    spec = NamedSharding(mesh, PartitionSpec("core"))
    concat_in = [
        jax.device_put(
            np.concatenate([in_maps[c][n] for c in range(NCORES)], axis=0),
            spec)
        for n in in_names
    ]

    def zeros():
        return [
            jax.device_put(
                np.zeros((NCORES * z.shape[0], *z.shape[1:]), z.dtype), spec)
            for z in zero_outs
        ]

    out_arrs = sharded(*concat_in, *zeros())  # warmup & result
    jax.block_until_ready(out_arrs)
    result = [np.asarray(a) for a in out_arrs]

    zsets = [zeros() for _ in range(iters)]
    jax.block_until_ready(zsets)
    t0 = time.perf_counter()
    last = None
    for zs in zsets:
        last = sharded(*concat_in, *zs)
    jax.block_until_ready(last)
    t1 = time.perf_counter()
    per_iter_ns = (t1 - t0) / iters * 1e9

    res_maps = [
        {n: result[i].reshape(NCORES, *out_avals[i].shape)[c]
         for i, n in enumerate(out_names)}
        for c in range(NCORES)
    ]

    class _R:
        results = res_maps

    return _assemble(_R), per_iter_ns


def _enable_ntff_hook():
    """Synthesize antenv.axon_hooks (absent in this image) and register the
    ctypes NTFF profile hook so run_bass_kernel_spmd(trace=True) works."""
    import sys as _sys
    import types as _types

    if "antenv.axon_hooks" in _sys.modules:
        return
    import antenv  # noqa: F401
    mod = _types.ModuleType("antenv.axon_hooks")
    mod._hook = None

    def set_axon_ntff_profile_hook(h):
        mod._hook = h

    def get_axon_ntff_profile_hook():
        return mod._hook

    mod.set_axon_ntff_profile_hook = set_axon_ntff_profile_hook
    mod.get_axon_ntff_profile_hook = get_axon_ntff_profile_hook
    _sys.modules["antenv.axon_hooks"] = mod
    antenv.axon_hooks = mod
    from trn_agent_boot.trn_boot import _ntff_profile_via_ctypes
    hook = _ntff_profile_via_ctypes("/opt/axon/libaxon_pjrt.so")
    if hook is not None:
        mod.set_axon_ntff_profile_hook(hook)
    # uploads need a fish bucket this container lacks; neuter them
    import concourse.bass_utils as _bu
    _bu.upload_artifacts = lambda tmpdir: f"local:{tmpdir}"


def kernel_traced(x, freqs_cos, freqs_sin, wq, wk, wv, wo, tmpdir=None):
    """Run once with NTFF tracing; returns (output, BassKernelResults)."""
    _enable_ntff_hook()
    in_maps = _host_prep(x, freqs_cos, freqs_sin, wq, wk, wv, wo)
    nc = _get_program()
    res = run_bass_kernel_spmd(
        nc, in_maps, core_ids=list(range(NCORES)), trace=True, tmpdir=tmpdir)
    return _assemble(res), res


# revision 62
# speedup vs baseline: 1.0948x; 1.0001x over previous
"""GQA causal attention (B=1, S=2048, D=4096, H=32, KV=8) on 8 trn2 cores.

Strategy: tensor-parallel over heads. Core i owns q-heads 4i..4i+3 and
kv-head i. Host pre-transposes weights/x so every matmul contracts along
the partition dim, and pre-permutes wq/wk rows (even|odd interleave ->
[evens;odds]) so RoPE becomes partition-aligned elementwise math.
Attention is computed head-locally in a scores^T [t, s] layout; softmax
sums are accumulated on the vector engine and reduced across partitions
with one gpsimd partition_all_reduce per (head, block) — no norm matmul
stream on the PE. Exp runs on 1024-wide pairs of score tiles to amortize
the activation engine's per-instruction overhead. After each 512-row
block of attention, the local out-projection shard is computed and a
bf16 ReduceScatter is fired as soon as each row-chunk completes, so the
collectives overlap compute. Host concatenates the per-core shards.

Matmul operands are bf16; accumulation, softmax and RoPE math are fp32.
"""

import sys

import numpy as np

sys.path.insert(0, "/opt/trn_rl_repo")

import ml_dtypes  # noqa: E402

import concourse.bass as bass  # noqa: E402
from concourse import bacc  # noqa: E402
from concourse import bass_isa  # noqa: E402
import concourse.mybir as mybir  # noqa: E402
import concourse.tile as tile  # noqa: E402
from concourse.bass_utils import run_bass_kernel_spmd  # noqa: E402

F32 = mybir.dt.float32
BF16 = mybir.dt.bfloat16
NPBF = ml_dtypes.bfloat16

B, S, D = 1, 2048, 4096
H, KV, HD = 32, 8, 128
NCORES = 8
HPC = H // NCORES  # q heads per core = 4
EQ = HPC * HD  # 512 local q features
NE = HPC + 2  # e-tiles per core: 4 q + 1 k + 1 v
SB = 512  # attention s block
NSB = S // SB  # 4
XSB = 512  # phase-1 s sub-block (matmul moving dim)
NXSB = S // XSB  # 4
DO = D // 128  # 32 contraction tiles for projections
TT = S // 128  # 16 t-tiles
RG = [list(range(NCORES))]
CH = [(0, 2), (2, 4), (12, 14), (14, 16), (8, 12), (4, 6), (6, 8)]


def build():
    nc = bacc.Bacc("TRN2", target_bir_lowering=False)
    # partition-major: [p, sb, do, c] so each phase-1 piece is a 16KB
    # contiguous run per partition (few, large DMA descriptors)
    xt = nc.dram_tensor("xt", [128, NXSB * DO * XSB], BF16,
                        kind="ExternalInput")
    # partition-major layout: [p, et, do, c] so each per-et DMA moves 8KB
    # contiguous per partition (line-rate) in PE consumption order
    wqkvt = nc.dram_tensor("wqkvt", [128, NE * DO * 128], BF16,
                           kind="ExternalInput")
    wot = nc.dram_tensor("wot", [EQ, D], BF16, kind="ExternalInput")
    cc = nc.dram_tensor("cc", [128, S], F32, kind="ExternalInput")
    ss = nc.dram_tensor("ss", [128, S], F32, kind="ExternalInput")
    masks2 = nc.dram_tensor("masks2", [2, 128, 2 * SB], BF16,
                            kind="ExternalInput")
    ident = nc.dram_tensor("ident", [128, 128], BF16, kind="ExternalInput")
    ones = nc.dram_tensor("ones", [128, 1], BF16, kind="ExternalInput")
    out = nc.dram_tensor("out", [NSB * 64, D], BF16, kind="ExternalOutput")

    xt_t = xt[:].rearrange("p (sb do c) -> p sb do c", sb=NXSB, do=DO)
    w_t = wqkvt[:].rearrange("p (et do c) -> p et (do c)", et=NE, do=DO)

    with tile.TileContext(nc) as tc:
        with tc.tile_pool(name="dram", bufs=1, space="DRAM") as dram, \
                tc.tile_pool(name="pqkv", bufs=1) as pqkv:
            rs_in = [dram.tile([(t1 - t0) * 128, D], BF16, name=f"rsi{ci}")
                     for ci, (t0, t1) in enumerate(CH)]
            rs_out = [dram.tile([(t1 - t0) * 16, D], BF16, name=f"rso{ci}")
                      for ci, (t0, t1) in enumerate(CH)]
            qe = [pqkv.tile([128, S], BF16, name=f"qe{et}", tag=f"qe{et}")
                  for et in range(NE)]
            # v in natural [t, hd] tiles (filled by per-sb PE transposes)
            vn = pqkv.tile([128, TT, HD], BF16, tag="vn")
            idt = pqkv.tile([128, 128], BF16, tag="idt")
            nc.scalar.dma_start(idt, ident[:])

            # ---------------- Phase 1: fused QKV projection + RoPE ----------
            with tc.tile_pool(name="p1w", bufs=1) as p1w, \
                    tc.tile_pool(name="p1x", bufs=2) as p1x, \
                    tc.tile_pool(name="p1t", bufs=1) as p1t, \
                    tc.tile_pool(name="p1ps", bufs=3, space="PSUM") as p1ps:
                w = p1w.tile([128, NE, DO, 128], BF16)
                # weights issued et-major so the PE's (sb0, et) groups are
                # fed in exactly the order they're consumed
                wv_ = w_t[:].rearrange("p et (do c) -> p et do c", do=DO)
                for dh in range(2):
                    dsl = slice(dh * (DO // 2), (dh + 1) * (DO // 2))
                    nc.scalar.dma_start(w[:, 0, dsl, :], wv_[:, 0, dsl, :])
                for et in range(1, NE):
                    nc.scalar.dma_start(
                        w[:, et].rearrange("p do c -> p (do c)"), w_t[:, et])
                # x for sb0/sb1 prefetched in pieces (first pieces smallest so
                # the PE's first accumulation group starts ASAP)
                xtiles, ctiles, stiles = [], [], []
                for sb in range(2):
                    ssl = slice(sb * XSB, (sb + 1) * XSB)
                    xtile = p1x.tile([128, DO, XSB], BF16, tag="x")
                    chunks = (8, 8, 8, 8) if sb == 0 else (16, 16)
                    d0 = 0
                    for nd in chunks:
                        dsl = slice(d0, d0 + nd)
                        nc.sync.dma_start(
                            xtile[:, dsl, :], xt_t[:, sb, dsl, :])
                        d0 += nd
                    cct = p1x.tile([128, XSB], F32, tag="cc")
                    sst = p1x.tile([128, XSB], F32, tag="ss")
                    nc.sync.dma_start(cct, cc[:][:, ssl])
                    nc.sync.dma_start(sst, ss[:][:, ssl])
                    xtiles.append(xtile)
                    ctiles.append(cct)
                    stiles.append(sst)
                for sb in range(NXSB):
                    ssl = slice(sb * XSB, (sb + 1) * XSB)
                    if sb < 2:
                        xtile, cct, sst = xtiles[sb], ctiles[sb], stiles[sb]
                    else:
                        xtile = p1x.tile([128, DO, XSB], BF16, tag="x")
                        nc.sync.dma_start(xtile, xt_t[:, sb])
                        cct = p1x.tile([128, XSB], F32, tag="cc")
                        sst = p1x.tile([128, XSB], F32, tag="ss")
                        nc.sync.dma_start(cct, cc[:][:, ssl])
                        nc.sync.dma_start(sst, ss[:][:, ssl])
                    atile = p1t.tile([128, HPC + 1, XSB], F32, tag="at")
                    btile = p1t.tile([128, HPC + 1, XSB], F32, tag="bt")
                    bsw = p1t.tile([128, (HPC + 1) * XSB], F32, tag="bsw")
                    bt2 = btile.rearrange("p e s -> p (e s)")
                    bs3 = bsw.rearrange("p (e s) -> p e s", s=XSB)
                    last = sb == NXSB - 1
                    # in the last sub-block, compute k first and finish its
                    # RoPE early so attention's first scores aren't gated on
                    # the whole RoPE tail
                    ets = (4, 0, 1, 2, 3, 5) if last else (0, 1, 2, 3, 4, 5)
                    for et in ets:
                        ps = p1ps.tile([128, XSB], F32, tag="ps")
                        for do in range(DO):
                            nc.tensor.matmul(
                                ps,
                                w[:, et, do, :],
                                xtile[:, do, :],
                                start=(do == 0),
                                stop=(do == DO - 1),
                            )
                        if et < HPC + 1:
                            # t*cos and t*sin halves; swap+add finishes RoPE
                            nc.vector.tensor_mul(atile[:, et, :], ps, cct)
                            nc.vector.tensor_mul(btile[:, et, :], ps, sst)
                            if last:
                                # per-et swap+add so each qe slice is ready
                                # ~2us after its matmuls, shrinking the
                                # phase-1 -> attention bubble
                                esl_ = slice(et * XSB, (et + 1) * XSB)
                                nc.gpsimd.dma_start(
                                    bsw[:64, esl_], bt2[64:, esl_])
                                nc.gpsimd.dma_start(
                                    bsw[64:, esl_], bt2[:64, esl_])
                                nc.vector.tensor_add(
                                    qe[et][:, ssl], atile[:, et, :],
                                    bs3[:, et, :])
                        else:
                            nc.scalar.copy(qe[et][:, ssl], ps)
                    if not last:
                        # half-swap of btile on the (idle) gpsimd DMA queue
                        nc.gpsimd.dma_start(bsw[:64, :], bt2[64:, :])
                        nc.gpsimd.dma_start(bsw[64:, :], bt2[:64, :])
                        for et in range(HPC + 1):
                            nc.vector.tensor_add(
                                qe[et][:, ssl], atile[:, et, :],
                                bs3[:, et, :])
                    # transpose this sb's four v tiles into [t, hd] layout
                    for i in range(4):
                        tt = 4 * sb + i
                        pst = p1ps.tile([128, 128], BF16, tag="tr", bufs=2)
                        nc.tensor.transpose(
                            pst, qe[HPC + 1][:, tt * 128:(tt + 1) * 128], idt)
                        nc.scalar.copy(vn[:, tt, :], pst)

            # ------- Phase 2+3: attention blocks + interleaved out-proj -----
            with tc.tile_pool(name="p3w", bufs=1) as p3w, \
                    tc.tile_pool(name="p2c", bufs=1) as p2c, \
                    tc.tile_pool(name="p2e", bufs=8) as p2e, \
                    tc.tile_pool(name="p2a", bufs=2) as p2a, \
                    tc.tile_pool(name="p2t", bufs=2) as p2t, \
                    tc.tile_pool(name="p3o", bufs=2) as p3o, \
                    tc.tile_pool(name="psS", bufs=2, space="PSUM") as psS, \
                    tc.tile_pool(name="pav", bufs=2, space="PSUM") as pavp, \
                    tc.tile_pool(name="pop", bufs=2, space="PSUM") as popp:
                mt = p2c.tile([128, 2, 2 * SB], BF16)
                nc.sync.dma_start(mt, masks2[:].rearrange("m p s -> p m s"))
                on = p2c.tile([128, 1], BF16)
                nc.sync.dma_start(on, ones[:])
                w3 = p3w.tile([128, HPC, D], BF16)
                nc.sync.dma_start(
                    w3, wot[:].rearrange("(eo p) d -> p eo d", p=128))
                ao = [p2c.tile([128, S], BF16, name=f"ao{h}", tag=f"ao{h}")
                      for h in range(HPC)]

                FIRED = {"tiles": set(), "fired": set()}

                def _flush(item):
                    fh, fbsl, fpau, fnrmb = item
                    nc.vector.tensor_mul(ao[fh][:, fbsl], fpau, fnrmb)

                def emit_attn(b):
                    bsl = slice(b * SB, (b + 1) * SB)
                    npair = 2 * b + 2
                    porder = list(range(npair))
                    deferred = []
                    for h in range(HPC):
                        # renorm of head h-2 runs now: its partition reduce is
                        # long done, so the DVE never head-of-line blocks
                        if len(deferred) >= 2:
                            _flush(deferred.pop(0))
                        pa = pavp.tile([128, SB], F32, tag="pav")
                        partials = []
                        for pi, p in enumerate(porder):
                            j0, j1 = 2 * p, 2 * p + 1
                            # diagonal tiles: columns < 128*m are masked for
                            # every partition, so narrow the matmuls to the
                            # live range. exp/mask read the stale (bounded)
                            # PSUM there and the mask zeroes it.
                            lo0 = max(0, j0 - 4 * b) * 128
                            lo1 = max(0, j1 - 4 * b) * 128
                            psc = psS.tile([128, 2 * SB], F32, tag="sc")
                            nc.tensor.matmul(
                                psc[:, lo0:SB],
                                qe[HPC][:, j0 * 128:(j0 + 1) * 128],
                                qe[h][:, b * SB + lo0:(b + 1) * SB],
                                start=True, stop=True)
                            nc.tensor.matmul(
                                psc[:, SB + lo1:],
                                qe[HPC][:, j1 * 128:(j1 + 1) * 128],
                                qe[h][:, b * SB + lo1:(b + 1) * SB],
                                start=True, stop=True)
                            ex = p2e.tile([128, 2 * SB], BF16, tag="ex")
                            nc.scalar.activation(
                                ex, psc, mybir.ActivationFunctionType.Exp)
                            if p >= 2 * b:
                                nc.vector.tensor_mul(
                                    ex, ex, mt[:, p - 2 * b, :])
                            # bf16 pair-sum feeding the softmax-denominator
                            # tree (2x DVE rate; depth<=4 so rounding is tiny)
                            tp = p2t.tile([128, SB], BF16, tag="tp", bufs=12)
                            nc.vector.tensor_add(tp, ex[:, :SB], ex[:, SB:])
                            partials.append(tp)
                            nc.tensor.matmul(
                                pa[:, lo0:], vn[:, j0, :], ex[:, lo0:SB],
                                start=(pi == 0), stop=False)
                            nc.tensor.matmul(
                                pa[:, lo1:], vn[:, j1, :], ex[:, SB + lo1:],
                                start=False, stop=(pi == npair - 1))
                        while len(partials) > 2:
                            nxt = []
                            for k in range(0, len(partials) - 1, 2):
                                u = p2t.tile([128, SB], BF16, tag="tp",
                                             bufs=12)
                                nc.vector.tensor_add(
                                    u, partials[k], partials[k + 1])
                                nxt.append(u)
                            if len(partials) % 2:
                                nxt.append(partials[-1])
                            partials = nxt
                        acc = p2a.tile([128, SB], BF16, tag="acc")
                        nc.vector.tensor_add(acc, partials[0], partials[1])
                        # partition-sum via a ones-matmul into a borrowed
                        # out-proj PSUM slot (idle during attention), then a
                        # cheap broadcast: ~2.5us less chain latency per head
                        # than gpsimd partition_all_reduce, and the gpsimd
                        # queue stays clear for the collective triggers
                        pn = popp.tile([128, SB], F32, tag="pop")
                        nc.tensor.matmul(pn[:1, :], on, acc,
                                         start=True, stop=True)
                        nrm1 = p2a.tile([1, SB], F32, tag="nrm1", bufs=4)
                        nc.vector.reciprocal_approx_fast(nrm1, pn[:1, :])
                        nrmb = p2a.tile([128, SB], F32, tag="nrm", bufs=4)
                        nc.gpsimd.partition_broadcast(nrmb, nrm1)
                        # evacuate pa so its PSUM bank recycles immediately
                        pau = p2a.tile([128, SB], F32, tag="pau", bufs=4)
                        nc.vector.tensor_copy(pau, pa)
                        deferred.append((h, bsl, pau, nrmb))
                    for item in deferred:
                        _flush(item)

                def emit_oproj(b):
                    # out-projection for this block's four s-tiles
                    done_tiles = FIRED["tiles"]
                    for stl_i in range(4):
                        st = b * 4 + stl_i
                        st0 = st * 128
                        ci = next(i for i, (t0, t1) in enumerate(CH)
                                  if t0 <= st < t1)
                        t0, t1 = CH[ci]
                        riv = rs_in[ci].rearrange("(t p) d -> p t d", p=128)
                        osb = p3o.tile([128, D], BF16, tag="osb")
                        for db in range(D // SB):
                            po = popp.tile([128, SB], F32, tag="pop")
                            for hh in range(HPC):
                                nc.tensor.matmul(
                                    po,
                                    ao[hh][:, st0:st0 + 128],
                                    w3[:, hh, db * SB:(db + 1) * SB],
                                    start=(hh == 0), stop=(hh == HPC - 1))
                            nc.scalar.copy(
                                osb[:, db * SB:(db + 1) * SB], po)
                        nc.sync.dma_start(riv[:, st - t0, :], osb)
                        done_tiles.add(st)
                        for ci2, (u0, u1) in enumerate(CH):
                            if ci2 not in FIRED["fired"] and all(
                                    t in done_tiles for t in range(u0, u1)):
                                FIRED["fired"].add(ci2)
                                nc.gpsimd.collective_compute(
                                    "ReduceScatter",
                                    mybir.AluOpType.add,
                                    ins=[rs_in[ci2].opt()],
                                    outs=[rs_out[ci2].opt()],
                                    replica_groups=RG,
                                )

                # Virtual ready-times pin the static schedule to this phase
                # order — without them the scheduler hoists out-proj
                # LDWEIGHTS/ship DMAs into earlier phases where their guards
                # stall the whole engine stream. Out-proj directly follows
                # each block so the ReduceScatter pipeline starts as early
                # as possible (it is the second-half critical path).
                # block 0 first: its attention is the shortest, so the RS
                # stream (the second-half critical path) starts ~35us
                # earlier; the last block's 4MB chunk is split so the kernel
                # ends on two small chunks
                emit_attn(0)
                with tc.tile_wait_until(1):
                    emit_oproj(0)
                with tc.tile_wait_until(2):
                    emit_attn(3)
                with tc.tile_wait_until(3):
                    emit_oproj(3)
                with tc.tile_wait_until(4):
                    emit_attn(2)
                with tc.tile_wait_until(5):
                    emit_oproj(2)
                with tc.tile_wait_until(6):
                    emit_attn(1)
                with tc.tile_wait_until(7):
                    emit_oproj(1)
                # tail: ship RS shards (bf16) straight to the output; host
                # upcasts to fp32
                # ship on the scalar HWDGE queue: the gpsimd queue is strict
                # FIFO and a ship DMA there would block the partition
                # reduces queued behind it for a whole collective
                orow = 0
                with tc.tile_wait_until(8):
                    for ci, (t0, t1) in enumerate(CH):
                        nr = (t1 - t0) * 16
                        nc.sync.dma_start(
                            out[:][orow:orow + nr, :], rs_out[ci][:])
                        orow += nr
    nc.compile()
    return nc


_CACHE = {}


def _get_program():
    if "nc" not in _CACHE:
        _CACHE["nc"] = build()
    return _CACHE["nc"]


def _host_prep(x, freqs_cos, freqs_sin, wq, wk, wv, wo):
    x2 = np.ascontiguousarray(np.asarray(x, np.float32).reshape(S, D))
    # partition-major repack: [p, sb, do, c] = xT[do*128+p, sb*512+c]
    xT = np.ascontiguousarray(
        x2.T.reshape(DO, 128, NXSB, XSB).transpose(1, 2, 0, 3)
        .reshape(128, NXSB * DO * XSB)).astype(NPBF)
    # even|odd -> [evens;odds] row permutation per head (RoPE partition split)
    perm1 = np.concatenate([np.arange(0, HD, 2), np.arange(1, HD, 2)])
    permq = (np.arange(H)[:, None] * HD + perm1[None, :]).reshape(-1)
    permk = (np.arange(KV)[:, None] * HD + perm1[None, :]).reshape(-1)
    scale = np.float32(1.0 / np.sqrt(HD))
    wq_p = np.asarray(wq, np.float32)[permq] * scale
    wk_p = np.asarray(wk, np.float32)[permk]
    wv32 = np.asarray(wv, np.float32)
    wo32 = np.asarray(wo, np.float32)
    cosT = np.asarray(freqs_cos, np.float32).T
    sinT = np.asarray(freqs_sin, np.float32).T
    ccb = np.ascontiguousarray(np.concatenate([cosT, cosT], 0))
    ssb = np.ascontiguousarray(np.concatenate([sinT, -sinT], 0))
    tp = np.arange(128, dtype=np.int64)[:, None]
    sf = np.arange(SB, dtype=np.int64)[None, :]
    masks = [(sf >= tp + 128 * m).astype(NPBF) for m in range(HPC)]
    masks2 = np.stack([np.concatenate([masks[0], masks[1]], 1),
                       np.concatenate([masks[2], masks[3]], 1)], 0)
    ident = np.eye(128, dtype=NPBF)
    ones_h = np.ones((128, 1), NPBF)

    in_maps = []
    for i in range(NCORES):
        wqkv = np.concatenate(
            [wq_p[i * EQ:(i + 1) * EQ],
             wk_p[i * HD:(i + 1) * HD],
             wv32[i * HD:(i + 1) * HD]], 0)  # [768, 4096]
        # partition-major repack: [p, et, do, c] with
        # element = wqkv[et*128 + c, do*128 + p]
        wqkvt = np.ascontiguousarray(
            wqkv.reshape(NE, 128, DO, 128).transpose(3, 0, 2, 1)
            .reshape(128, NE * DO * 128)).astype(NPBF)
        wot = np.ascontiguousarray(
            wo32[:, i * EQ:(i + 1) * EQ].T).astype(NPBF)  # [512, 4096]
        in_maps.append(dict(xt=xT, wqkvt=wqkvt, wot=wot, cc=ccb, ss=ssb,
                            masks2=masks2, ident=ident, ones=ones_h))
    return in_maps


def _run(in_maps, trace=False):
    nc = _get_program()
    return run_bass_kernel_spmd(
        nc, in_maps, core_ids=list(range(NCORES)), trace=trace)


CH_HOST = list(CH)


def _assemble(res):
    full = np.empty((S, D), np.float32)
    for r in range(NCORES):
        shard = np.asarray(res.results[r]["out"]).astype(np.float32)
        orow = 0
        for (t0, t1) in CH_HOST:
            nr = (t1 - t0) * 16
            full[t0 * 128 + r * nr: t0 * 128 + (r + 1) * nr, :] = \
                shard[orow:orow + nr, :]
            orow += nr
    return full.reshape(B, S, D)


def kernel(x, freqs_cos, freqs_sin, wq, wk, wv, wo):
    in_maps = _host_prep(x, freqs_cos, freqs_sin, wq, wk, wv, wo)
    res = _run(in_maps, trace=False)
    return _assemble(res)


def _build_sharded():
    """Mirror of bass2jax.run_bass_via_pjrt's multi-core path, split so the
    jitted callable and device-resident inputs can be reused for timing."""
    import jax
    from jax.experimental.shard_map import shard_map
    from jax.sharding import Mesh, PartitionSpec

    import concourse.mybir as mb
    from concourse import bass2jax

    nc = _get_program()
    bass2jax.install_neuronx_cc_hook()
    part_name = (nc.partition_id_tensor.name
                 if nc.partition_id_tensor else None)
    in_names, out_names, out_avals, zero_outs = [], [], [], []
    for alloc in nc.m.functions[0].allocations:
        if not isinstance(alloc, mb.MemoryLocationSet):
            continue
        name = alloc.memorylocations[0].name
        if alloc.kind == "ExternalInput":
            if name != part_name:
                in_names.append(name)
        elif alloc.kind == "ExternalOutput":
            out_names.append(name)
            shape = tuple(alloc.tensor_shape)
            dtype = mb.dt.np(alloc.dtype)
            out_avals.append(jax.core.ShapedArray(shape, dtype))
            zero_outs.append(np.zeros(shape, dtype))
    n_params = len(in_names)
    all_names = in_names + out_names
    if part_name is not None:
        all_names = all_names + [part_name]

    def _body(*args):
        operands = list(args)
        if part_name is not None:
            operands.append(bass2jax.partition_id_tensor())
        outs = bass2jax._bass_exec_p.bind(
            *operands,
            out_avals=tuple(out_avals),
            in_names=tuple(all_names),
            out_names=tuple(out_names),
            lowering_input_output_aliases=(),
            sim_require_finite=True,
            sim_require_nnan=True,
            nc=nc,
        )
        return tuple(outs)

    devices = jax.devices()[:NCORES]
    mesh = Mesh(np.asarray(devices), ("core",))
    n_outs = len(out_names)
    sharded = jax.jit(
        shard_map(
            _body, mesh=mesh,
            in_specs=(PartitionSpec("core"),) * (n_params + n_outs),
            out_specs=(PartitionSpec("core"),) * n_outs,
            check_rep=False,
        ),
        donate_argnums=tuple(range(n_params, n_params + n_outs)),
        keep_unused=True,
    )
    return sharded, in_names, out_names, out_avals, zero_outs, mesh


def kernel_profiled(x, freqs_cos, freqs_sin, wq, wk, wv, wo, iters=12):
    """Returns (output, per-execution wall ns). Times repeated on-device
    executions with inputs pre-placed on the devices."""
    import time

    import jax
    from jax.sharding import NamedSharding, PartitionSpec

    in_maps = _host_prep(x, freqs_cos, freqs_sin, wq, wk, wv, wo)
    sharded, in_names, out_names, out_avals, zero_outs, mesh = _build_sharded()
    spec = NamedSharding(mesh, PartitionSpec("core"))
    concat_in = [
        jax.device_put(
            np.concatenate([in_maps[c][n] for c in range(NCORES)], axis=0),
            spec)
        for n in in_names
    ]

    def zeros():
        return [
            jax.device_put(
                np.zeros((NCORES * z.shape[0], *z.shape[1:]), z.dtype), spec)
            for z in zero_outs
        ]

    out_arrs = sharded(*concat_in, *zeros())  # warmup & result
    jax.block_until_ready(out_arrs)
    result = [np.asarray(a) for a in out_arrs]

    zsets = [zeros() for _ in range(iters)]
    jax.block_until_ready(zsets)
    t0 = time.perf_counter()
    last = None
    for zs in zsets:
        last = sharded(*concat_in, *zs)
    jax.block_until_ready(last)
    t1 = time.perf_counter()
    per_iter_ns = (t1 - t0) / iters * 1e9

    res_maps = [
        {n: result[i].reshape(NCORES, *out_avals[i].shape)[c]
         for i, n in enumerate(out_names)}
        for c in range(NCORES)
    ]

    class _R:
        results = res_maps

    return _assemble(_R), per_iter_ns


def _enable_ntff_hook():
    """Synthesize antenv.axon_hooks (absent in this image) and register the
    ctypes NTFF profile hook so run_bass_kernel_spmd(trace=True) works."""
    import sys as _sys
    import types as _types

    if "antenv.axon_hooks" in _sys.modules:
        return
    import antenv  # noqa: F401
    mod = _types.ModuleType("antenv.axon_hooks")
    mod._hook = None

    def set_axon_ntff_profile_hook(h):
        mod._hook = h

    def get_axon_ntff_profile_hook():
        return mod._hook

    mod.set_axon_ntff_profile_hook = set_axon_ntff_profile_hook
    mod.get_axon_ntff_profile_hook = get_axon_ntff_profile_hook
    _sys.modules["antenv.axon_hooks"] = mod
    antenv.axon_hooks = mod
    from trn_agent_boot.trn_boot import _ntff_profile_via_ctypes
    hook = _ntff_profile_via_ctypes("/opt/axon/libaxon_pjrt.so")
    if hook is not None:
        mod.set_axon_ntff_profile_hook(hook)
    # uploads need a fish bucket this container lacks; neuter them
    import concourse.bass_utils as _bu
    _bu.upload_artifacts = lambda tmpdir: f"local:{tmpdir}"


def kernel_traced(x, freqs_cos, freqs_sin, wq, wk, wv, wo, tmpdir=None):
    """Run once with NTFF tracing; returns (output, BassKernelResults)."""
    _enable_ntff_hook()
    in_maps = _host_prep(x, freqs_cos, freqs_sin, wq, wk, wv, wo)
    nc = _get_program()
    res = run_bass_kernel_spmd(
        nc, in_maps, core_ids=list(range(NCORES)), trace=True, tmpdir=tmpdir)
    return _assemble(res), res
